# revision 1
# baseline (speedup 1.0000x reference)
"""DiM block (Mamba-style selective-scan transformer block) on 8 TRN2 cores.

Sharding: core i handles (b = i//4, k = i%4) — one batch sample and one of
the 4 scan directions. The spatial permutation q_k (identity / transpose /
flip / anti-transpose) is pushed onto host-prepared inputs (x pre-permuted,
conv taps transformed) so ONE SPMD program serves all 8 cores. The per-sample
combine over k happens on-device via an AllGather within each 4-core group,
followed by compile-time (uniform) re-orderings.

Scan layout: partitions = (n-pair:2 outer, d:64 inner), free = t.
Per (d-group, n-pair) tile: PE broadcasts -dt via a selector matmul, ACT
applies exp with the per-partition A column (dA = exp(A*dt)), DVE/GPSIMD run
multiply / hardware-scan / multiply, PE reduces the n-pair into a PSUM y
accumulator.
"""
import json
import sys

sys.path.insert(0, "/opt/trn_rl_repo")

import numpy as np
import concourse.bass as bass
import concourse.mybir as mybir
import concourse.tile as tile
from concourse.bass_utils import run_bass_kernel_spmd

# ---------------------------------------------------------------------------
# Workaround: this walrus build rejects instructions carrying >1 embedded
# sem-wait. Split extra waits onto same-engine NoOps at BIR serialization.
_MAXW = 1
_wsplit_counter = [0]


def _split_multi_waits(bir: dict) -> dict:
    for fn in bir.get("functions", []):
        for bb in fn.get("blocks", []):
            insts = bb.get("instructions", [])
            if not any(
                len((i.get("sync_info") or {}).get("on_wait") or []) > _MAXW
                for i in insts
            ):
                continue
            out = []
            for inst in insts:
                si = inst.get("sync_info")
                waits = (si or {}).get("on_wait") or []
                if len(waits) > _MAXW and inst.get("engine"):
                    for w in waits[:-_MAXW]:
                        _wsplit_counter[0] += 1
                        out.append({
                            "debug": inst.get("debug", 0),
                            "engine": inst["engine"],
                            "ins": [], "outs": [],
                            "name": f"I-wsplit-{_wsplit_counter[0]}",
                            "opcode": "NoOp",
                            "sync_info": {"on_update": [], "on_wait": [w]},
                        })
                    si["on_wait"] = waits[-_MAXW:]
                out.append(inst)
            bb["instructions"] = out
    return bir


_orig_to_json_bytes = bass.Bass.to_json_bytes


def _patched_to_json_bytes(self) -> bytes:
    j = json.loads(_orig_to_json_bytes(self))
    _split_multi_waits(j)
    return json.dumps(j).encode()


bass.Bass.to_json_bytes = _patched_to_json_bytes

# ---------------------------------------------------------------------------
B, Hs, Ws, DIM = 2, 32, 32, 256
L = Hs * Ws
DI = 2 * DIM
DS = 64
DTR = DIM // 16
K = 4
HID = 4 * DIM

f32 = mybir.dt.float32
bf16 = mybir.dt.bfloat16
MUL = mybir.AluOpType.mult
ADD = mybir.AluOpType.add
SUB = mybir.AluOpType.subtract
BYP = mybir.AluOpType.bypass
AF = mybir.ActivationFunctionType
AX = mybir.AxisListType

EPS = 1e-6
NPAIRS = DS // 2          # 32 n-pairs
CHUNK = 4                 # n-pairs per chunk
NCHUNK = NPAIRS // CHUNK  # 4


def build_program(scan_on_gpsimd=False, xin_mod=8, yc_gps_mod=0):
    nc = bass.Bass()

    def din(name, shape, dt=f32):
        return nc.dram_tensor(name, list(shape), dt, kind="ExternalInput")

    T = {}
    T["x_pre"] = din("x_pre", (L, DIM))
    T["xT_pre"] = din("xT_pre", (DIM, L))
    T["x_row"] = din("x_row", (L, DIM))
    T["xT_row"] = din("xT_row", (DIM, L))
    T["c_vec"] = din("c_vec", (1, DIM))
    T["W_ada"] = din("W_ada", (DIM, 6 * DIM))
    T["b_ada"] = din("b_ada", (1, 6 * DIM))
    T["W_in_xi"] = din("W_in_xi", (DIM, DI))
    T["W_in_z"] = din("W_in_z", (DIM, DI))
    T["b_in_xi"] = din("b_in_xi", (128, 4))
    T["b_in_z"] = din("b_in_z", (128, 4))
    T["convw"] = din("convw", (128, 36))      # [d_part, dtile*9 + tap]
    T["convb"] = din("convb", (128, 4))
    T["W_xp"] = din("W_xp", (DI, 144))        # cols reordered [B(64), C(64), dtr(16)]
    T["W_dtm"] = din("W_dtm", (DTR, DI))
    T["dtb"] = din("dtb", (128, 4))
    T["a_logcols"] = din("a_logcols", (128, 256))  # [p, g*32+i] = A_log[k, g*64+p%64, 2i+p//64]
    T["sel2"] = din("sel2", (2, 128, 128))    # [par][k,p] = (k == p%64 + par*64)
    T["ysel"] = din("ysel", (128, 64))        # [p, d] = (p%64 == d)
    T["Dp_c"] = din("Dp_c", (128, 4))
    T["lnw_c"] = din("lnw_c", (128, 4))
    T["lnb_c"] = din("lnb_c", (128, 4))
    T["W_out"] = din("W_out", (DI, DIM))
    T["b_out_c"] = din("b_out_c", (128, 2))
    T["W_fc1"] = din("W_fc1", (DIM, HID))
    T["b_fc1_c"] = din("b_fc1_c", (128, 8))
    T["W_fc2"] = din("W_fc2", (HID, DIM))
    T["b_fc2_c"] = din("b_fc2_c", (128, 2))
    T["ident"] = din("ident", (128, 128))

    T["outT"] = nc.dram_tensor("outT", [DIM, L], f32, kind="ExternalOutput")
    T["ys_la"] = nc.dram_tensor("ys_la", [DI, L], bf16)
    T["ys_lb"] = nc.dram_tensor("ys_lb", [DI, L], bf16)
    T["ys_ga"] = nc.dram_tensor("ys_ga", [4, DI, L], bf16)
    T["ys_gb"] = nc.dram_tensor("ys_gb", [4, DI, L], bf16)
    T["mod_scr"] = nc.dram_tensor("mod_scr", [1792], f32)
    T["z_spill"] = nc.dram_tensor("z_spill", [DI, L], bf16)

    with tile.TileContext(nc) as tc:
        _build_body(nc, tc, T, scan_on_gpsimd, xin_mod, yc_gps_mod)
    return nc


def _build_body(nc, tc, T, scan_on_gpsimd, xin_mod=5, yc_gps_mod=0):
    from contextlib import ExitStack

    dma = nc.sync.dma_start

    perstack = ExitStack()
    persist = perstack.enter_context(tc.tile_pool(name="persist", bufs=1))
    wstack = ExitStack()
    wp = wstack.enter_context(tc.tile_pool(name="weights", bufs=1))
    work = wstack.enter_context(tc.tile_pool(name="work", bufs=1))
    pre_ps = ExitStack()
    pp = pre_ps.enter_context(tc.tile_pool(name="ps_pre", bufs=2, space="PSUM"))

    # ---------------- early x loads (these gate the pre-phase chain) ------
    xstat_tiles = {}
    tl = []
    for j in range(8):
        xt = work.tile([128, DIM], f32, tag="lnx", name=f"lnx_pre{j}", bufs=8)
        dma(xt[:], T["x_pre"][j * 128:(j + 1) * 128, :])
        tl.append(xt)
    xstat_tiles["x_pre"] = tl

    # ---------------- S0: load + cast weights (staged through 2 slots) ----
    def load_cast(dram, rows, cols, name):
        n = (rows + 127) // 128
        outs = []
        for j in range(n):
            r0 = j * 128
            r = min(128, rows - r0)
            tf = work.tile([r, cols], f32, tag="stage", name=f"st_{name}{j}", bufs=2)
            dma(tf[:], dram[r0:r0 + r, :])
            tb = wp.tile([r, cols], bf16, tag=f"{name}_b{j}", name=f"{name}_b{j}")
            nc.gpsimd.tensor_copy(tb[:], tf[:])
            outs.append(tb)
        return outs

    Wxi = load_cast(T["W_in_xi"], DIM, DI, "Wxi")          # 2 x (128,512)
    Wz = load_cast(T["W_in_z"], DIM, DI, "Wz")
    Wxp = load_cast(T["W_xp"], DI, 144, "Wxp")             # 4 x (128,144)
    Wdt = load_cast(T["W_dtm"], DTR, DI, "Wdt")            # 1 x (16,512)
    Wada = []
    for j in range(2):
        tb = wp.tile([128, 6 * DIM], bf16, tag=f"Wada_b{j}", name=f"Wada_b{j}")
        for hcol in range(2):
            tf = work.tile([128, 1024], f32, tag="stage", name=f"st_Wada{j}_{hcol}", bufs=2)
            dma(tf[:, :768], T["W_ada"][j * 128:(j + 1) * 128, hcol * 768:(hcol + 1) * 768])
            nc.vector.tensor_copy(tb[:, hcol * 768:(hcol + 1) * 768], tf[:, :768])
        Wada.append(tb)
    sel_b = []
    for par in range(2):
        tf = work.tile([128, 128], f32, tag="stage", name=f"st_sel{par}", bufs=2)
        dma(tf[:], T["sel2"][par, :, :])
        tb = wp.tile([128, 128], bf16, tag=f"sel_b{par}", name=f"sel_b{par}")
        nc.vector.tensor_copy(tb[:], tf[:])
        sel_b.append(tb)
    ysel_b = load_cast(T["ysel"], 128, 64, "ysel")[0]

    identf = persist.tile([128, 128], f32, tag="identf", name="identf")
    dma(identf[:], T["ident"][:, :])

    smalls = {}
    for nm in ("b_in_xi", "b_in_z", "convw", "convb", "dtb", "Dp_c",
               "lnw_c", "lnb_c", "b_out_c", "b_fc1_c", "b_fc2_c"):
        tt = persist.tile(list(T[nm].shape), f32, tag=nm, name=nm)
        dma(tt[:], T[nm][:, :])
        smalls[nm] = tt

    eps_col = persist.tile([128, 1], f32, tag="eps_col", name="eps_col")
    nc.gpsimd.memset(eps_col[:], EPS)
    alog = work.tile([128, 256], f32, tag="stage", name="alog", bufs=2)
    dma(alog[:], T["a_logcols"][:, :])
    acols = wp.tile([128, 256], f32, tag="acols", name="acols")
    nc.scalar.activation(acols[:], alog[:], AF.Exp)

    # ---------------- S1: adaLN modulation vector -------------------------
    c_t = work.tile([1, DIM], f32, tag="c_t", name="c_t")
    dma(c_t[:], T["c_vec"][:, :])
    c_silu = work.tile([1, DIM], f32, tag="c_silu", name="c_silu")
    nc.scalar.activation(c_silu[:], c_t[:], AF.Silu)
    c_col = work.tile([128, 2], f32, tag="c_col", name="c_col")
    dma(T["mod_scr"][1536:1792], c_silu[0:1, :])
    dma(c_col[:], T["mod_scr"][1536:1792].rearrange("(j p) -> p j", j=2, p=128))
    c_colb = work.tile([128, 2], bf16, tag="c_colb", name="c_colb")
    nc.vector.tensor_copy(c_colb[:], c_col[:])

    mod = work.tile([1, 6 * DIM], f32, tag="mod", name="mod")
    for fb in range(3):
        bada_s = work.tile([1, 512], f32, tag="bada_s", name=f"bada_s{fb}", bufs=2)
        dma(bada_s[:], T["b_ada"][0:1, fb * 512:(fb + 1) * 512])
        pmod = pp.tile([1, 512], f32, tag="pmod", name=f"pmod{fb}", bufs=1)
        for kk in range(2):
            nc.tensor.matmul(pmod[:], c_colb[:, kk:kk + 1],
                             Wada[kk][:, fb * 512:(fb + 1) * 512],
                             start=(kk == 0), stop=(kk == 1))
        nc.vector.tensor_tensor(mod[:, fb * 512:(fb + 1) * 512], pmod[:],
                                bada_s[:], ADD)
        dma(T["mod_scr"][fb * 512:(fb + 1) * 512], mod[0:1, fb * 512:(fb + 1) * 512])

    mcols = []
    for i6 in range(6):
        col = persist.tile([128, 2], f32, tag=f"mcol{i6}", name=f"mcol{i6}")
        dma(col[:], T["mod_scr"][i6 * 256:(i6 + 1) * 256]
            .rearrange("(j p) -> p j", j=2, p=128))
        mcols.append(col)
    sh_msa, sc_msa, g_msa, sh_mlp, sc_mlp, g_mlp = mcols
    s1_msa = persist.tile([128, 2], f32, tag="s1_msa", name="s1_msa")
    nc.scalar.activation(s1_msa[:], sc_msa[:], AF.Identity, bias=1.0)
    s1_mlp = persist.tile([128, 2], f32, tag="s1_mlp", name="s1_mlp")
    nc.scalar.activation(s1_mlp[:], sc_mlp[:], AF.Identity, bias=1.0)
    gb_out = persist.tile([128, 2], f32, tag="gb_out", name="gb_out")
    nc.vector.tensor_tensor(gb_out[:], g_msa[:], smalls["b_out_c"][:], MUL)
    gb_fc2 = persist.tile([128, 2], f32, tag="gb_fc2", name="gb_fc2")
    nc.vector.tensor_tensor(gb_fc2[:], g_mlp[:], smalls["b_fc2_c"][:], MUL)

    # ---------------- shared helpers -------------------------------------
    def ln_stats(xtiles, name, pool, x_dram=None):
        mu = pool.tile([128, 8], f32, tag=f"mu_{name}", name=f"mu_{name}")
        sq = pool.tile([128, 8], f32, tag=f"sq_{name}", name=f"sq_{name}")
        for j in range(8):
            if xtiles is None:
                xt = pool.tile([128, DIM], f32, tag="lnxr", name=f"lnxr_{name}{j}", bufs=2)
                dma(xt[:], x_dram[j * 128:(j + 1) * 128, :])
            else:
                xt = xtiles[j]
            nc.vector.tensor_reduce(mu[:, j:j + 1], xt[:], AX.X, ADD)
            scr = pool.tile([128, DIM], f32, tag="lnscr", name=f"lnscr_{name}{j}", bufs=2)
            nc.scalar.activation(scr[:], xt[:], AF.Square, accum_out=sq[:, j:j + 1])
        nc.vector.tensor_scalar_mul(mu[:], mu[:], 1.0 / DIM)
        mu2 = pool.tile([128, 8], f32, tag="ln_mu2", name=f"mu2_{name}")
        nc.vector.tensor_tensor(mu2[:], mu[:], mu[:], MUL)
        var = pool.tile([128, 8], f32, tag="ln_var", name=f"var_{name}")
        nc.vector.scalar_tensor_tensor(var[:], sq[:], 1.0 / DIM, mu2[:], MUL, SUB)
        std = pool.tile([128, 8], f32, tag="ln_std", name=f"std_{name}")
        nc.scalar.activation(std[:], var[:], AF.Sqrt, bias=eps_col[:, 0:1])
        rstd = pool.tile([128, 8], f32, tag=f"rstd_{name}", name=f"rstd_{name}")
        nc.vector.reciprocal(rstd[:], std[:])
        return mu, rstd

    def bcast_cols(stat, name, pool, psum_pool, tag):
        """(128,8) per-token stat -> (128,1024) all-partition broadcast tile."""
        statT_p = psum_pool.tile([8, 128], f32, tag="statT_p", name=f"sTp_{name}", bufs=1)
        nc.tensor.transpose(statT_p[:], stat[:], identf[:])
        statT = pool.tile([8, 128], f32, tag="statT", name=f"sT_{name}", bufs=2)
        nc.scalar.copy(statT[:], statT_p[:])
        row2 = pool.tile([2, L], f32, tag="row2", name=f"r2_{name}", bufs=1)
        dma(row2[0:1, :], statT[:, :])
        dma(row2[1:2, :], statT[:, :])
        bc = pool.tile([128, L], f32, tag=tag, name=f"bc_{name}", bufs=1)
        dma(bc[:], row2[:, :].partition_broadcast(64).rearrange("n d f -> d n f"))
        return bc

    def make_hT(xT_dram, mu, rstd, name, pool, psum_pool):
        mu_bc = bcast_cols(mu, f"mu{name}", pool, psum_pool, "bcA")
        rstd_bc = bcast_cols(rstd, f"rs{name}", pool, psum_pool, "bcB")
        outs = []
        for cc in range(2):
            xt = pool.tile([128, L], f32, tag="hscr", name=f"hx_{name}{cc}", bufs=2)
            dma(xt[:], xT_dram[cc * 128:(cc + 1) * 128, :])
            t1 = pool.tile([128, L], f32, tag="hscr", name=f"hs1_{name}{cc}", bufs=2)
            nc.vector.tensor_tensor(t1[:], xt[:], mu_bc[:], SUB)
            t2 = pool.tile([128, L], f32, tag="hscr", name=f"hs2_{name}{cc}", bufs=2)
            nc.vector.tensor_tensor(t2[:], t1[:], rstd_bc[:], MUL)
            hb = pool.tile([128, L], bf16, tag=f"hTb_{name}{cc}", name=f"hTb_{name}{cc}")
            nc.scalar.activation(hb[:], t2[:], AF.Identity,
                                 bias=sh_msa[:, cc:cc + 1], scale=s1_msa[:, cc:cc + 1])
            outs.append(hb)
        return outs

    # ---------------- S2: LN1 + modulate (pre and row domains) ------------
    mu_p, rstd_p = ln_stats(xstat_tiles["x_pre"], "p", work)
    hT_pre = make_hT(T["xT_pre"], mu_p, rstd_p, "p", work, pp)
    mu_r, rstd_r = ln_stats(None, "r", work, x_dram=T["x_row"])
    hT_row = make_hT(T["xT_row"], mu_r, rstd_r, "r", work, pp)

    # ---------------- S3: xi / z projections ------------------------------
    def proj(hT, Wk, bias, name, dest_pool, tag_unique):
        outs = []
        for j in range(4):
            ppm = pp.tile([128, L], f32, tag="projp", name=f"pp_{name}{j}", bufs=2)
            for kk in range(2):
                for th in range(2):
                    nc.tensor.matmul(
                        ppm[:, th * 512:(th + 1) * 512],
                        Wk[kk][:, j * 128:(j + 1) * 128],
                        hT[kk][:, th * 512:(th + 1) * 512],
                        start=(kk == 0), stop=(kk == 1))
            if tag_unique:
                ot = dest_pool.tile([128, L], bf16, tag=f"{name}{j}", name=f"{name}{j}")
            else:
                ot = dest_pool.tile([128, L], bf16, tag=name, name=f"{name}{j}", bufs=2)
            nc.scalar.activation(ot[:], ppm[:], AF.Identity, bias=bias[:, j:j + 1])
            outs.append(ot)
        return outs

    xiT = proj(hT_pre, Wxi, smalls["b_in_xi"], "xiT", work, False)
    zT_tiles = proj(hT_row, Wz, smalls["b_in_z"], "zTs", work, False)
    for j in range(4):
        dma(T["z_spill"][j * 128:(j + 1) * 128, :], zT_tiles[j][:])

    # ---------------- S4: depthwise conv 3x3 + silu -----------------------
    xsT = []
    for j in range(4):
        pad = work.tile([128, 34 * 34], bf16, tag="pad", name=f"pad{j}", bufs=2)
        nc.gpsimd.memset(pad[:], 0.0)
        pad3 = pad[:, :].rearrange("p (H W) -> p H W", H=34, W=34)
        src = xiT[j][:, :].rearrange("p (h w) -> p h w", h=32, w=32)
        nc.vector.tensor_copy(pad3[:, 1:33, 1:33], src)
        acc = None
        for tap in range(9):
            dy, dx = tap // 3, tap % 3
            sh = pad3[:, dy:dy + 32, dx:dx + 32]
            wcol = smalls["convw"][:, j * 9 + tap:j * 9 + tap + 1]
            if acc is None:
                acc = work.tile([128, L], bf16, tag="cacc", name=f"cacc{j}_0", bufs=2)
                nc.scalar.mul(acc[:].rearrange("p (h w) -> p h w", h=32, w=32),
                              sh, wcol)
            else:
                acc2 = work.tile([128, L], bf16, tag="cacc", name=f"cacc{j}_{tap}", bufs=2)
                nc.vector.scalar_tensor_tensor(
                    acc2[:].rearrange("p (h w) -> p h w", h=32, w=32),
                    sh, wcol, acc[:].rearrange("p (h w) -> p h w", h=32, w=32),
                    MUL, ADD)
                acc = acc2
        xs = wp.tile([128, L], bf16, tag=f"xsT{j}", name=f"xsT{j}")
        nc.scalar.activation(xs[:], acc[:], AF.Silu, bias=smalls["convb"][:, j:j + 1])
        xsT.append(xs)

    # ---------------- S5: x_dbl = [B; C] and dt_r -------------------------
    bc_t = wp.tile([128, L], bf16, tag="bc_t", name="bc_t")
    ppbc = pp.tile([128, L], f32, tag="projp", name="ppbc", bufs=2)
    for kk in range(4):
        for th in range(2):
            nc.tensor.matmul(ppbc[:, th * 512:(th + 1) * 512],
                             Wxp[kk][:, 0:128],
                             xsT[kk][:, th * 512:(th + 1) * 512],
                             start=(kk == 0), stop=(kk == 3))
    nc.scalar.copy(bc_t[:], ppbc[:])
    dtr_t = wp.tile([16, L], bf16, tag="dtr_t", name="dtr_t")
    ppdtr = pp.tile([16, L], f32, tag="ppdtr", name="ppdtr", bufs=1)
    for kk in range(4):
        for th in range(2):
            nc.tensor.matmul(ppdtr[:, th * 512:(th + 1) * 512],
                             Wxp[kk][:, 128:144],
                             xsT[kk][:, th * 512:(th + 1) * 512],
                             start=(kk == 0), stop=(kk == 3))
    nc.scalar.copy(dtr_t[:], ppdtr[:])

    # ---------------- S6: dt, -dt, w --------------------------------------
    negdtT, wT, wbc = [], [], []
    for j in range(4):
        ppd = pp.tile([128, L], f32, tag="projp", name=f"ppdt{j}", bufs=2)
        for th in range(2):
            nc.tensor.matmul(ppd[:, th * 512:(th + 1) * 512],
                             Wdt[0][:, j * 128:(j + 1) * 128],
                             dtr_t[:, th * 512:(th + 1) * 512],
                             start=True, stop=True)
        spx = work.tile([128, L], f32, tag="spx", name=f"spx{j}", bufs=2)
        nc.scalar.activation(spx[:], ppd[:], AF.Exp, bias=smalls["dtb"][:, j:j + 1])
        dt_b = work.tile([128, L], bf16, tag="dtT", name=f"dtT{j}", bufs=2)
        nc.scalar.activation(dt_b[:], spx[:], AF.Ln, bias=1.0)
        ndt = wp.tile([128, L], bf16, tag=f"ndtT{j}", name=f"ndtT{j}")
        nc.vector.tensor_scalar_mul(ndt[:], dt_b[:], -1.0)
        negdtT.append(ndt)
        w_b = work.tile([128, L], bf16, tag="wTtmp", name=f"wT{j}", bufs=1)
        nc.vector.tensor_tensor(w_b[:], dt_b[:], xsT[j][:], MUL)
        wT.append(w_b)
        for par in range(2):
            g = 2 * j + par
            wsrc = w_b[par * 64:par * 64 + 64, :]
            wb = wp.tile([128, L], bf16, tag=f"wbc{g}", name=f"wbc{g}")
            dma(wb[0:64, :], wsrc)
            dma(wb[64:128, :], wsrc)
            wbc.append(wb)

    pre_ps.close()

    # ---------------- S7/S8: scan stage -----------------------------------
    y_sb = [None] * 4
    scan_st = ExitStack()
    argp = scan_st.enter_context(tc.tile_pool(name="argp", bufs=3, space="PSUM"))
    yps = scan_st.enter_context(tc.tile_pool(name="yps", bufs=1, space="PSUM"))
    spool = scan_st.enter_context(tc.tile_pool(name="spool", bufs=2))
    bpool = scan_st.enter_context(tc.tile_pool(name="bpool", bufs=1))
    ypool = scan_st.enter_context(tc.tile_pool(name="ypool", bufs=1))

    it = 0
    for ch in range(NCHUNK):
        Bb, Cb = [], []
        for il in range(CHUNK):
            i = ch * CHUNK + il
            bbt = bpool.tile([128, L], bf16, tag="Bb", name=f"Bb{ch}_{il}", bufs=CHUNK)
            dma(bbt[:], bc_t[2 * i:2 * i + 2, :]
                .partition_broadcast(64).rearrange("d n f -> n d f"))
            Bb.append(bbt)
            cbt = bpool.tile([128, L], bf16, tag="Cb", name=f"Cb{ch}_{il}", bufs=CHUNK)
            dma(cbt[:], bc_t[64 + 2 * i:64 + 2 * i + 2, :]
                .partition_broadcast(64).rearrange("d n f -> n d f"))
            Cb.append(cbt)
        for j in range(4):
            ypt = yps.tile([128, L], f32, tag="ypt", name=f"ypt{ch}_{j}")
            for par in range(2):
                g = 2 * j + par
                for il in range(CHUNK):
                    i = ch * CHUNK + il
                    arg = argp.tile([128, L], f32, tag="arg", name=f"arg{it}")
                    for th in range(2):
                        nc.tensor.matmul(arg[:, th * 512:(th + 1) * 512], sel_b[par][:],
                                         negdtT[j][:, th * 512:(th + 1) * 512],
                                         start=True, stop=True)
                    dA = spool.tile([128, L], bf16, tag="dA", name=f"dA{it}", bufs=2)
                    nc.scalar.activation(dA[:], arg[:], AF.Exp,
                                         scale=acols[:, g * 32 + i:g * 32 + i + 1])
                    xin = spool.tile([128, L], bf16, tag="xin", name=f"xin{it}", bufs=4)
                    xeng = nc.gpsimd if (xin_mod and it % xin_mod) else nc.vector
                    xeng.tensor_tensor(xin[:], wbc[g][:], Bb[il][:], MUL)
                    h = spool.tile([128, L], bf16, tag="h", name=f"h{it}", bufs=3)
                    seng = nc.gpsimd if scan_on_gpsimd else nc.vector
                    seng.tensor_tensor_scan(h[:], dA[:], xin[:], 0.0, MUL, ADD)
                    yc = spool.tile([128, L], bf16, tag="yc", name=f"yc{it}", bufs=3)
                    yeng = nc.gpsimd if (yc_gps_mod and it % yc_gps_mod == 0) else nc.vector
                    yeng.tensor_tensor(yc[:], h[:], Cb[il][:], MUL)
                    for th in range(2):
                        nc.tensor.matmul(
                            ypt[par * 64:par * 64 + 64, th * 512:(th + 1) * 512],
                            ysel_b[:], yc[:, th * 512:(th + 1) * 512],
                            start=(il == 0), stop=(il == CHUNK - 1))
                    it += 1
            if ch == 0:
                ynew = ypool.tile([128, L], f32, tag="ysb", name=f"ysb{j}_0", bufs=5)
                nc.scalar.copy(ynew[:], ypt[:])
                y_sb[j] = ynew
            else:
                ynew = ypool.tile([128, L], f32, tag="ysb", name=f"ysb{j}_{ch}", bufs=5)
                nc.vector.tensor_tensor(ynew[:], ypt[:], y_sb[j][:], ADD)
                y_sb[j] = ynew

    # ---------------- S9: ys = y + Dp*xs -> DRAM (bf16) -------------------
    for j in range(4):
        ysb16 = spool.tile([128, L], bf16, tag="ys16", name=f"ys16_{j}", bufs=2)
        nc.vector.scalar_tensor_tensor(ysb16[:], xsT[j][:],
                                       smalls["Dp_c"][:, j:j + 1], y_sb[j][:],
                                       MUL, ADD)
        dma(T["ys_lb"][j * 128:(j + 1) * 128, :], ysb16[:])

    scan_st.close()
    wstack.close()

    post = ExitStack()
    pf = post.enter_context(tc.tile_pool(name="postf", bufs=1))
    ppost = post.enter_context(tc.tile_pool(name="ps_post", bufs=2, space="PSUM"))

    # collective-independent post work, emitted first so it can overlap the
    # AllGather window
    ones_f = pf.tile([128, 1], f32, tag="ones_f", name="ones_f")
    nc.gpsimd.memset(ones_f[:], 1.0)
    xTr = []
    for cc in range(2):
        xt = pf.tile([128, L], f32, tag=f"xTr{cc}", name=f"xTr{cc}")
        dma(xt[:], T["xT_row"][cc * 128:(cc + 1) * 128, :])
        xTr.append(xt)
    siluz = []
    for j in range(4):
        zrl = pf.tile([128, L], bf16, tag="zrl", name=f"zrl{j}", bufs=2)
        dma(zrl[:], T["z_spill"][j * 128:(j + 1) * 128, :])
        sz = pf.tile([128, L], bf16, tag="siluz", name=f"siluz{j}", bufs=4)
        nc.scalar.activation(sz[:], zrl[:], AF.Silu)
        siluz.append(sz)

    def load_cast_pf(dram, rows, cols, name):
        n = (rows + 127) // 128
        outs = []
        for j in range(n):
            r0 = j * 128
            r = min(128, rows - r0)
            tf = pf.tile([r, cols], f32, tag="pstage", name=f"pst_{name}{j}", bufs=2)
            dma(tf[:], dram[r0:r0 + r, :])
            tb = pf.tile([r, cols], bf16, tag=f"{name}_b{j}", name=f"{name}_b{j}")
            nc.vector.tensor_copy(tb[:], tf[:])
            outs.append(tb)
        return outs

    Wout = load_cast_pf(T["W_out"], DI, DIM, "Wout")
    Wfc1 = load_cast_pf(T["W_fc1"], DIM, HID, "Wfc1")
    Wfc2 = load_cast_pf(T["W_fc2"], HID, DIM, "Wfc2")

    # ---------------- S10: AllGather --------------------------------------
    nc.gpsimd.collective_compute(
        "AllGather", BYP,
        replica_groups=[[0, 1, 2, 3], [4, 5, 6, 7]],
        ins=[T["ys_lb"][:, :]],
        outs=[T["ys_gb"][:, :, :]],
    )

    # ---------------- S11: combine the 4 directions -----------------------
    y_row = []
    for j in range(4):
        ysk = []
        for k in range(4):
            t16 = pf.tile([128, L], bf16, tag="ysk", name=f"ysk{j}_{k}", bufs=5)
            dma(t16[:], T["ys_gb"][k, j * 128:(j + 1) * 128, :])
            ysk.append(t16)
        # k=3: materialize the reversed copy, then treat like k=1
        rev3 = pf.tile([128, L], bf16, tag="rev3", name=f"rev3_{j}", bufs=2)
        nc.vector.tensor_copy(rev3[:], ysk[3][:, ::-1])
        acc = pf.tile([128, L], f32, tag="yrow", name=f"yrow{j}_0", bufs=5)
        nc.scalar.copy(acc[:], ysk[0][:])
        for k in (1, 2, 3):
            nacc = pf.tile([128, L], f32, tag="yrow", name=f"yrow{j}_{k}", bufs=5)
            ceng = nc.gpsimd if (j % 2 == 0) else nc.vector
            if k == 2:
                ceng.tensor_tensor(nacc[:], acc[:], ysk[2][:, ::-1], ADD)
            else:
                srct = ysk[1] if k == 1 else rev3
                view = (srct[:, :].rearrange("p (w h) -> p w h", w=32, h=32)
                                  .rearrange("p w h -> p h w"))
                ceng.tensor_tensor(
                    nacc[:].rearrange("p (h w) -> p h w", h=32, w=32),
                    acc[:].rearrange("p (h w) -> p h w", h=32, w=32),
                    view, ADD)
            acc = nacc
        y_row.append(acc)

    # ---------------- S12: LN over d + ln_w/ln_b, gate --------------------
    def part_stats(tiles, ntiles, name):
        ssum = ppost.tile([128, 8], f32, tag="ssum", name=f"ssum_{name}", bufs=1)
        ssq = ppost.tile([128, 8], f32, tag="ssq", name=f"ssq_{name}", bufs=1)
        sqt = []
        for j in range(ntiles):
            sq = pf.tile([128, L], f32, tag="sqt", name=f"sq_{name}{j}", bufs=4)
            nc.scalar.activation(sq[:], tiles[j][:], AF.Square)
            sqt.append(sq)
        for tb in range(8):
            for j in range(ntiles):
                nc.tensor.matmul(ssum[:, tb:tb + 1],
                                 tiles[j][:, tb * 128:(tb + 1) * 128], ones_f[:],
                                 start=(j == 0), stop=(j == ntiles - 1))
                nc.tensor.matmul(ssq[:, tb:tb + 1],
                                 sqt[j][:, tb * 128:(tb + 1) * 128], ones_f[:],
                                 start=(j == 0), stop=(j == ntiles - 1))
        return ssum, ssq

    def stats_to_bcast(ssum, ssq, dim, name):
        mu = pf.tile([128, 8], f32, tag="pmu", name=f"pmu_{name}", bufs=2)
        nc.vector.tensor_scalar_mul(mu[:], ssum[:], 1.0 / dim)
        mu2 = pf.tile([128, 8], f32, tag="pmu2", name=f"pmu2_{name}", bufs=2)
        nc.vector.tensor_tensor(mu2[:], mu[:], mu[:], MUL)
        var = pf.tile([128, 8], f32, tag="pvar", name=f"pvar_{name}", bufs=2)
        nc.vector.scalar_tensor_tensor(var[:], ssq[:], 1.0 / dim, mu2[:], MUL, SUB)
        std = pf.tile([128, 8], f32, tag="pstd", name=f"pstd_{name}", bufs=2)
        nc.scalar.activation(std[:], var[:], AF.Sqrt, bias=eps_col[:, 0:1])
        rstd = pf.tile([128, 8], f32, tag="prstd", name=f"prstd_{name}", bufs=2)
        nc.vector.reciprocal(rstd[:], std[:])
        mu_bc = bcast_cols(mu, f"P{name}m", pf, ppost, "bcA")
        rstd_bc = bcast_cols(rstd, f"P{name}r", pf, ppost, "bcB")
        return mu_bc, rstd_bc

    ysum, ysq = part_stats(y_row, 4, "y")
    ymu_bc, yrstd_bc = stats_to_bcast(ysum, ysq, DI, "y")

    gated = []
    for j in range(4):
        peng = nc.gpsimd if (j % 2 == 0) else nc.vector
        t1 = pf.tile([128, L], f32, tag="psc", name=f"lny1_{j}", bufs=3)
        peng.tensor_tensor(t1[:], y_row[j][:], ymu_bc[:], SUB)
        t2 = pf.tile([128, L], f32, tag="psc", name=f"lny2_{j}", bufs=3)
        peng.tensor_tensor(t2[:], t1[:], yrstd_bc[:], MUL)
        t3 = pf.tile([128, L], f32, tag="psc", name=f"lny3_{j}", bufs=3)
        nc.scalar.activation(t3[:], t2[:], AF.Identity,
                             bias=smalls["lnb_c"][:, j:j + 1],
                             scale=smalls["lnw_c"][:, j:j + 1])
        gt = pf.tile([128, L], bf16, tag=f"gated{j}", name=f"gated{j}")
        peng.tensor_tensor(gt[:], t3[:], siluz[j][:], MUL)
        gated.append(gt)

    # ---------------- S14: hy, residual -----------------------------------
    x2T = []
    for cc in range(2):
        php = ppost.tile([128, L], f32, tag="php", name=f"php{cc}", bufs=2)
        for kk in range(4):
            for th in range(2):
                nc.tensor.matmul(php[:, th * 512:(th + 1) * 512],
                                 Wout[kk][:, cc * 128:(cc + 1) * 128],
                                 gated[kk][:, th * 512:(th + 1) * 512],
                                 start=(kk == 0), stop=(kk == 3))
        t1 = pf.tile([128, L], f32, tag="psc", name=f"hyg{cc}", bufs=3)
        nc.scalar.activation(t1[:], php[:], AF.Identity,
                             bias=gb_out[:, cc:cc + 1], scale=g_msa[:, cc:cc + 1])
        x2 = pf.tile([128, L], f32, tag=f"x2T{cc}", name=f"x2T{cc}")
        nc.vector.tensor_tensor(x2[:], t1[:], xTr[cc][:], ADD)
        x2T.append(x2)

    # ---------------- S15: LN2 + modulate ---------------------------------
    x2sum, x2sq = part_stats(x2T, 2, "x2")
    x2mu_bc, x2rstd_bc = stats_to_bcast(x2sum, x2sq, DIM, "x2")
    mT = []
    for cc in range(2):
        qeng = nc.gpsimd if (cc % 2 == 0) else nc.vector
        t1 = pf.tile([128, L], f32, tag="psc", name=f"m1_{cc}", bufs=3)
        qeng.tensor_tensor(t1[:], x2T[cc][:], x2mu_bc[:], SUB)
        t2 = pf.tile([128, L], f32, tag="psc", name=f"m2_{cc}", bufs=3)
        qeng.tensor_tensor(t2[:], t1[:], x2rstd_bc[:], MUL)
        mb = pf.tile([128, L], bf16, tag=f"mT{cc}", name=f"mT{cc}")
        nc.scalar.activation(mb[:], t2[:], AF.Identity,
                             bias=sh_mlp[:, cc:cc + 1], scale=s1_mlp[:, cc:cc + 1])
        mT.append(mb)

    # ---------------- S16: MLP + final residual ---------------------------
    gelu = []
    for j in range(8):
        pfc = ppost.tile([128, L], f32, tag="php", name=f"pfc1_{j}", bufs=2)
        for kk in range(2):
            for th in range(2):
                nc.tensor.matmul(pfc[:, th * 512:(th + 1) * 512],
                                 Wfc1[kk][:, j * 128:(j + 1) * 128],
                                 mT[kk][:, th * 512:(th + 1) * 512],
                                 start=(kk == 0), stop=(kk == 1))
        gl = pf.tile([128, L], bf16, tag=f"gelu{j}", name=f"gelu{j}")
        nc.scalar.activation(gl[:], pfc[:], AF.Gelu_apprx_tanh, bias=smalls["b_fc1_c"][:, j:j + 1])
        gelu.append(gl)

    for cc in range(2):
        pfc2 = ppost.tile([128, L], f32, tag="php", name=f"pfc2_{cc}", bufs=2)
        for kk in range(8):
            for th in range(2):
                nc.tensor.matmul(pfc2[:, th * 512:(th + 1) * 512],
                                 Wfc2[kk][:, cc * 128:(cc + 1) * 128],
                                 gelu[kk][:, th * 512:(th + 1) * 512],
                                 start=(kk == 0), stop=(kk == 7))
        t1 = pf.tile([128, L], f32, tag="psc", name=f"mlpg{cc}", bufs=3)
        nc.scalar.activation(t1[:], pfc2[:], AF.Identity,
                             bias=gb_fc2[:, cc:cc + 1], scale=g_mlp[:, cc:cc + 1])
        o = pf.tile([128, L], f32, tag="outTt", name=f"outT{cc}", bufs=2)
        nc.vector.tensor_tensor(o[:], t1[:], x2T[cc][:], ADD)
        dma(T["outT"][cc * 128:(cc + 1) * 128, :], o[:])

    post.close()
    perstack.close()


# ---------------------------------------------------------------------------
# Host side
_PROGRAM = None


def _get_program():
    global _PROGRAM
    if _PROGRAM is None:
        _PROGRAM = build_program()
    return _PROGRAM


def _q_img(x, k):
    img = x.reshape(Hs, Ws, -1)
    if k == 0:
        out = img
    elif k == 1:
        out = img.transpose(1, 0, 2)
    elif k == 2:
        out = img[::-1, ::-1]
    else:
        out = img.transpose(1, 0, 2)[::-1, ::-1]
    return np.ascontiguousarray(out.reshape(L, -1))


def _conv_w_q(w, k):
    if k == 0:
        return w
    if k == 1:
        return np.ascontiguousarray(w.transpose(1, 0, 2))
    if k == 2:
        return np.ascontiguousarray(w[::-1, ::-1])
    return np.ascontiguousarray(w.transpose(1, 0, 2)[::-1, ::-1])


def _col128(v, ncols):
    return np.ascontiguousarray(v.reshape(ncols, 128).T)


def prep_inputs(inputs):
    inp = {k: np.asarray(v, dtype=np.float32) for k, v in inputs.items()}
    x, c = inp["x"], inp["c"]

    shared = {}
    shared["W_ada"] = inp["W_ada"]
    shared["b_ada"] = inp["b_ada"].reshape(1, 6 * DIM)
    W_in = inp["W_in"]
    shared["W_in_xi"] = np.ascontiguousarray(W_in[:, :DI])
    shared["W_in_z"] = np.ascontiguousarray(W_in[:, DI:])
    b_in = inp["b_in"]
    shared["b_in_xi"] = _col128(b_in[:DI], 4)
    shared["b_in_z"] = _col128(b_in[DI:], 4)
    shared["lnw_c"] = _col128(inp["ln_w"], 4)
    shared["lnb_c"] = _col128(inp["ln_b"], 4)
    shared["W_out"] = inp["W_out"]
    shared["b_out_c"] = _col128(inp["b_out"], 2)
    shared["W_fc1"] = inp["W_fc1"]
    shared["b_fc1_c"] = _col128(inp["b_fc1"], 8)
    shared["W_fc2"] = inp["W_fc2"]
    shared["b_fc2_c"] = _col128(inp["b_fc2"], 2)
    shared["ident"] = np.eye(128, dtype=np.float32)
    p = np.arange(128)
    sel2 = np.zeros((2, 128, 128), np.float32)
    for par in range(2):
        sel2[par, p % 64 + par * 64, p] = 1.0
    shared["sel2"] = sel2
    ys = np.zeros((128, 64), np.float32)
    ys[p, p % 64] = 1.0
    shared["ysel"] = ys

    in_maps = []
    for core in range(8):
        b, k = core // 4, core % 4
        m = dict(shared)
        xb = x[b]
        xpre = _q_img(xb, k)
        m["x_pre"] = xpre
        m["xT_pre"] = np.ascontiguousarray(xpre.T)
        m["x_row"] = np.ascontiguousarray(xb)
        m["xT_row"] = np.ascontiguousarray(xb.T)
        m["c_vec"] = c[b].reshape(1, DIM)

        cw = _conv_w_q(inp["conv_w"].reshape(3, 3, DI), k)
        cwt = cw.reshape(9, 4, 128)                       # [tap, dtile, p]
        m["convw"] = np.ascontiguousarray(cwt.transpose(2, 1, 0).reshape(128, 36))
        m["convb"] = _col128(inp["conv_b"], 4)

        Wxp = inp["W_xproj"][k]                           # (DI, 144) cols [dtr,B,C]
        m["W_xp"] = np.ascontiguousarray(
            np.concatenate([Wxp[:, DTR:DTR + DS], Wxp[:, DTR + DS:], Wxp[:, :DTR]],
                           axis=1))
        m["W_dtm"] = np.ascontiguousarray(inp["W_dt"][k])
        m["dtb"] = _col128(inp["dt_bias"][k], 4)
        m["Dp_c"] = _col128(inp["Dp"][k], 4)

        alog = inp["A_log"][k]                            # (DI, DS)
        acols = np.zeros((128, 256), np.float32)
        for g in range(8):
            for i in range(NPAIRS):
                acols[:, g * 32 + i] = alog[g * 64 + (p % 64), 2 * i + (p // 64)]
        m["a_logcols"] = acols
        in_maps.append(m)
    return in_maps


def kernel(**inputs):
    nc = _get_program()
    in_maps = prep_inputs(inputs)
    res = run_bass_kernel_spmd(nc, in_maps, list(range(8)))
    out = np.zeros((B, L, DIM), np.float32)
    for b in range(B):
        out[b] = res.results[4 * b]["outT"].T
    return out



# revision 81
# speedup vs baseline: 1.2021x; 1.2021x over previous
"""DiM block (Mamba-style selective-scan transformer block) on 8 TRN2 cores.

Sharding: core i handles (b = i//4, k = i%4) — one batch sample and one of
the 4 scan directions. The spatial permutation q_k (identity / transpose /
flip / anti-transpose) is pushed onto host-prepared inputs (x pre-permuted,
conv taps transformed) so ONE SPMD program serves all 8 cores. The per-sample
combine over k happens on-device via per-d-group AllGathers within each
4-core group (chunked so they overlap the scan), followed by compile-time
(uniform) re-orderings done as identity-matmuls with permuted moving APs.

Scan layout: partitions = (n-pair:2 outer, d:64 inner), free = t.
Per (d-group g, n-pair i): ACT computes dA = exp(A*dt) from a pre-broadcast
-dt tile, DVE does the w*B and h*C multiplies (bf16 2x mode), the hardware
scan runs mostly on GPSIMD (1/8 on DVE to balance), PE reduces the n-pair
into a per-d-group PSUM y accumulator that lives across the whole group.
"""
import json
import sys

sys.path.insert(0, "/opt/trn_rl_repo")

import numpy as np
import concourse.bass as bass
import concourse.mybir as mybir
import concourse.tile as tile
from concourse.bass_utils import run_bass_kernel_spmd

# ---------------------------------------------------------------------------
# Workaround: this walrus build rejects instructions carrying >1 embedded
# sem-wait. Split extra waits onto same-engine NoOps at BIR serialization.
_MAXW = 1
_wsplit_counter = [0]


def _split_multi_waits(bir: dict) -> dict:
    for fn in bir.get("functions", []):
        for bb in fn.get("blocks", []):
            insts = bb.get("instructions", [])
            if not any(
                len((i.get("sync_info") or {}).get("on_wait") or []) > _MAXW
                for i in insts
            ):
                continue
            out = []
            for inst in insts:
                si = inst.get("sync_info")
                waits = (si or {}).get("on_wait") or []
                if len(waits) > _MAXW and inst.get("engine"):
                    for w in waits[:-_MAXW]:
                        _wsplit_counter[0] += 1
                        out.append({
                            "debug": inst.get("debug", 0),
                            "engine": inst["engine"],
                            "ins": [], "outs": [],
                            "name": f"I-wsplit-{_wsplit_counter[0]}",
                            "opcode": "NoOp",
                            "sync_info": {"on_update": [], "on_wait": [w]},
                        })
                    si["on_wait"] = waits[-_MAXW:]
                out.append(inst)
            bb["instructions"] = out
    return bir


_orig_to_json_bytes = bass.Bass.to_json_bytes


def _patched_to_json_bytes(self) -> bytes:
    j = json.loads(_orig_to_json_bytes(self))
    _split_multi_waits(j)
    return json.dumps(j).encode()


bass.Bass.to_json_bytes = _patched_to_json_bytes

# ---------------------------------------------------------------------------
B, Hs, Ws, DIM = 2, 32, 32, 256
L = Hs * Ws
DI = 2 * DIM
DS = 64
DTR = DIM // 16
K = 4
HID = 4 * DIM

f32 = mybir.dt.float32
bf16 = mybir.dt.bfloat16
MUL = mybir.AluOpType.mult
ADD = mybir.AluOpType.add
SUB = mybir.AluOpType.subtract
BYP = mybir.AluOpType.bypass
AF = mybir.ActivationFunctionType
AX = mybir.AxisListType

EPS = 1e-6
NPAIRS = DS // 2          # 32 n-pairs
CHUNK = 4                 # n-pairs per chunk
NCHUNK = NPAIRS // CHUNK  # 8
PADW = 34 * 34 + 8        # padded conv plane + slack for shifted windows
CFREE = 1088              # conv output plane (32 rows x 34 cols)


def build_program(dve_scan_mod=0, pe_combine=True):
    nc = bass.Bass()

    def din(name, shape, dt=f32):
        return nc.dram_tensor(name, list(shape), dt, kind="ExternalInput")

    T = {}
    T["x_pre"] = din("x_pre", (L, DIM))
    T["xT_pre"] = din("xT_pre", (DIM, L))
    T["x_row"] = din("x_row", (L, DIM))
    T["xT_row"] = din("xT_row", (DIM, L))
    T["c_vec"] = din("c_vec", (1, DIM))
    T["W_ada"] = din("W_ada", (DIM, 6 * DIM))
    T["b_ada"] = din("b_ada", (1, 6 * DIM))
    T["W_in_xi"] = din("W_in_xi", (DIM, DI))
    T["W_in_z"] = din("W_in_z", (DIM, DI))
    T["b_in_xi"] = din("b_in_xi", (128, 4))
    T["b_in_z"] = din("b_in_z", (128, 4))
    T["convw"] = din("convw", (128, 36))      # [d_part, dtile*9 + tap]
    T["convb"] = din("convb", (128, 4))
    T["W_xp"] = din("W_xp", (DI, 144))        # cols reordered [B(64), C(64), dtr(16)]
    T["W_dtm"] = din("W_dtm", (DTR, DI))
    T["dtb"] = din("dtb", (128, 4))
    T["a_logcols"] = din("a_logcols", (128, 256))  # [p, g*32+i] = A_log[k, g*64+p%64, 2i+p//64]
    T["ysel"] = din("ysel", (128, 64))        # [p, d] = (p%64 == d)
    T["Dp_c"] = din("Dp_c", (128, 4))
    T["lnw_c"] = din("lnw_c", (128, 4))
    T["lnb_c"] = din("lnb_c", (128, 4))
    T["W_out"] = din("W_out", (DI, DIM))
    T["b_out_c"] = din("b_out_c", (128, 2))
    T["W_fc1"] = din("W_fc1", (DIM, HID))
    T["b_fc1_c"] = din("b_fc1_c", (128, 8))
    T["W_fc2"] = din("W_fc2", (HID, DIM))
    T["b_fc2_c"] = din("b_fc2_c", (128, 2))
    T["ident"] = din("ident", (128, 128))

    T["outT"] = nc.dram_tensor("outT", [DIM, L], f32, kind="ExternalOutput")
    T["ys_lb"] = nc.dram_tensor("ys_lb", [DI, L], bf16)
    for g in range(8):
        T[f"ys_g{g}"] = nc.dram_tensor(f"ys_g{g}", [4, 64, L], bf16)
    T["z_spill"] = nc.dram_tensor("z_spill", [DI, L], bf16)

    with tile.TileContext(nc) as tc:
        _build_body(nc, tc, T, dve_scan_mod, pe_combine)
    return nc


def _build_body(nc, tc, T, dve_scan_mod, pe_combine):
    from contextlib import ExitStack

    dma = nc.sync.dma_start

    perstack = ExitStack()
    persist = perstack.enter_context(tc.tile_pool(name="persist", bufs=1))
    wstack = ExitStack()
    wp = wstack.enter_context(tc.tile_pool(name="weights", bufs=1))
    workstack = ExitStack()
    work = workstack.enter_context(tc.tile_pool(name="work", bufs=1))
    pre_ps = ExitStack()
    pp = pre_ps.enter_context(tc.tile_pool(name="ps_pre", bufs=2, space="PSUM"))

    # ---------------- early x loads (these gate the pre-phase chain) ------
    xstat_tiles = {}
    tl = []
    for j in range(8):
        xt = work.tile([128, DIM], f32, tag="lnx", name=f"lnx_pre{j}", bufs=8)
        dma(xt[:], T["x_pre"][j * 128:(j + 1) * 128, :])
        tl.append(xt)
    xstat_tiles["x_pre"] = tl

    # ---------------- S0: load + cast weights (staged through 2 slots) ----
    def load_cast(dram, rows, cols, name, dest=None):
        dest = dest or work
        n = (rows + 127) // 128
        outs = []
        for j in range(n):
            r0 = j * 128
            r = min(128, rows - r0)
            tf = work.tile([r, cols], f32, tag="stage", name=f"st_{name}{j}", bufs=2)
            dma(tf[:], dram[r0:r0 + r, :])
            tb = dest.tile([r, cols], bf16, tag=f"{name}_b{j}", name=f"{name}_b{j}")
            nc.gpsimd.tensor_copy(tb[:], tf[:])
            outs.append(tb)
        return outs

    identf = persist.tile([128, 128], f32, tag="identf", name="identf")
    dma(identf[:], T["ident"][:, :])
    identb = persist.tile([128, 128], bf16, tag="identb", name="identb")
    nc.gpsimd.tensor_copy(identb[:], identf[:])

    Wada = []
    for j in range(2):
        tb = work.tile([128, 6 * DIM], bf16, tag=f"Wada_b{j}", name=f"Wada_b{j}")
        for hcol in range(2):
            tf = work.tile([128, 1024], f32, tag="stage", name=f"st_Wada{j}_{hcol}", bufs=2)
            dma(tf[:, :768], T["W_ada"][j * 128:(j + 1) * 128, hcol * 768:(hcol + 1) * 768])
            nc.vector.tensor_copy(tb[:, hcol * 768:(hcol + 1) * 768], tf[:, :768])
        Wada.append(tb)
    eps_col = persist.tile([128, 1], f32, tag="eps_col", name="eps_col")
    nc.gpsimd.memset(eps_col[:], EPS)
    # ---------------- S1: adaLN modulation vector -------------------------
    c_t = work.tile([1, DIM], f32, tag="c_t", name="c_t")
    dma(c_t[:], T["c_vec"][:, :])
    c_silu = work.tile([1, DIM], f32, tag="c_silu", name="c_silu")
    nc.scalar.activation(c_silu[:], c_t[:], AF.Silu)
    c_colp = pp.tile([128, 2], f32, tag="projp", name="c_colp", bufs=2)
    for half in range(2):
        nc.tensor.transpose(c_colp[:, half:half + 1],
                            c_silu[0:1, half * 128:(half + 1) * 128],
                            identf[0:1, 0:1])
    c_colb = work.tile([128, 2], bf16, tag="c_colb", name="c_colb")
    nc.scalar.copy(c_colb[:], c_colp[:])

    mod = work.tile([1, 6 * DIM], f32, tag="mod", name="mod")
    for fb in range(3):
        bada_s = work.tile([1, 512], f32, tag="bada_s", name=f"bada_s{fb}", bufs=2)
        dma(bada_s[:], T["b_ada"][0:1, fb * 512:(fb + 1) * 512])
        pmod = pp.tile([1, 512], f32, tag="projp", name=f"pmod{fb}", bufs=2)
        for kk in range(2):
            nc.tensor.matmul(pmod[:], c_colb[:, kk:kk + 1],
                             Wada[kk][:, fb * 512:(fb + 1) * 512],
                             start=(kk == 0), stop=(kk == 1))
        nc.vector.tensor_tensor(mod[:, fb * 512:(fb + 1) * 512], pmod[:],
                                bada_s[:], ADD)

    mcols = []
    for i6 in range(6):
        mcp = pp.tile([128, 2], f32, tag="projp", name=f"mcolp{i6}", bufs=2)
        for half in range(2):
            nc.tensor.transpose(
                mcp[:, half:half + 1],
                mod[0:1, i6 * 256 + half * 128:i6 * 256 + (half + 1) * 128],
                identf[0:1, 0:1])
        col = persist.tile([128, 2], f32, tag=f"mcol{i6}", name=f"mcol{i6}")
        nc.scalar.copy(col[:], mcp[:])
        mcols.append(col)
    sh_msa, sc_msa, g_msa, sh_mlp, sc_mlp, g_mlp = mcols
    s1_msa = persist.tile([128, 2], f32, tag="s1_msa", name="s1_msa")
    nc.scalar.activation(s1_msa[:], sc_msa[:], AF.Identity, bias=1.0)
    s1_mlp = persist.tile([128, 2], f32, tag="s1_mlp", name="s1_mlp")
    nc.scalar.activation(s1_mlp[:], sc_mlp[:], AF.Identity, bias=1.0)

    # ---------------- shared helpers -------------------------------------
    def ln_stats(xtiles, name, pool, x_dram=None):
        mu = pool.tile([128, 8], f32, tag=f"mu_{name}", name=f"mu_{name}")
        sq = pool.tile([128, 8], f32, tag=f"sq_{name}", name=f"sq_{name}")
        for j in range(8):
            if xtiles is None:
                xt = pool.tile([128, DIM], f32, tag="lnxr", name=f"lnxr_{name}{j}", bufs=2)
                dma(xt[:], x_dram[j * 128:(j + 1) * 128, :])
            else:
                xt = xtiles[j]
            nc.vector.tensor_reduce(mu[:, j:j + 1], xt[:], AX.X, ADD)
            scr = pool.tile([128, DIM], f32, tag="lnscr", name=f"lnscr_{name}{j}", bufs=2)
            nc.scalar.activation(scr[:], xt[:], AF.Square, accum_out=sq[:, j:j + 1])
        nc.vector.tensor_scalar_mul(mu[:], mu[:], 1.0 / DIM)
        mu2 = pool.tile([128, 8], f32, tag="ln_mu2", name=f"mu2_{name}")
        nc.vector.tensor_tensor(mu2[:], mu[:], mu[:], MUL)
        var = pool.tile([128, 8], f32, tag="ln_var", name=f"var_{name}")
        nc.vector.scalar_tensor_tensor(var[:], sq[:], 1.0 / DIM, mu2[:], MUL, SUB)
        std = pool.tile([128, 8], f32, tag="ln_std", name=f"std_{name}")
        nc.scalar.activation(std[:], var[:], AF.Sqrt, bias=eps_col[:, 0:1])
        rstd = pool.tile([128, 8], f32, tag=f"rstd_{name}", name=f"rstd_{name}")
        nc.vector.reciprocal(rstd[:], std[:])
        return mu, rstd

    def bcast_cols(stat, name, pool, tag, psum_pool, ptag, dt_out=f32):
        """(128,8) per-token stat -> (128,1024) all-partition broadcast tile."""
        statT_p = psum_pool.tile([8, 128], f32, tag=ptag, name=f"sTp_{name}", bufs=2)
        nc.tensor.transpose(statT_p[:], stat[:], identf[:])
        statT = pool.tile([8, 128], dt_out, tag="statT", name=f"sT_{name}", bufs=2)
        nc.scalar.copy(statT[:], statT_p[:])
        row2 = pool.tile([2, L], dt_out, tag=f"row2{dt_out}", name=f"r2_{name}", bufs=2)
        dma(row2[0:1, :], statT[:, :])
        dma(row2[1:2, :], statT[:, :])
        bc = pool.tile([128, L], dt_out, tag=tag, name=f"bc_{name}", bufs=1)
        dma(bc[:], row2[:, :].partition_broadcast(64).rearrange("n d f -> d n f"))
        return bc

    def make_hT(xT_dram, mu, rstd, name, pool):
        mu_bc = bcast_cols(mu, f"mu{name}", pool, "bcA", pp, "projp")
        rstd_bc = bcast_cols(rstd, f"rs{name}", pool, "bcB", pp, "projp")
        outs = []
        for cc in range(2):
            xt = pool.tile([128, L], f32, tag="hscr", name=f"hx_{name}{cc}", bufs=2)
            dma(xt[:], xT_dram[cc * 128:(cc + 1) * 128, :])
            t1 = pool.tile([128, L], f32, tag="hscr", name=f"hs1_{name}{cc}", bufs=2)
            nc.vector.tensor_tensor(t1[:], xt[:], mu_bc[:], SUB)
            t2 = pool.tile([128, L], f32, tag="hscr", name=f"hs2_{name}{cc}", bufs=2)
            nc.vector.tensor_tensor(t2[:], t1[:], rstd_bc[:], MUL)
            hb = pool.tile([128, L], bf16, tag=f"hTb_{name}{cc}", name=f"hTb_{name}{cc}")
            nc.scalar.activation(hb[:], t2[:], AF.Identity,
                                 bias=sh_msa[:, cc:cc + 1], scale=s1_msa[:, cc:cc + 1])
            outs.append(hb)
        return outs

    # ---------------- S2: LN1 + modulate (pre and row domains) ------------
    mu_p, rstd_p = ln_stats(xstat_tiles["x_pre"], "p", work)
    hT_pre = make_hT(T["xT_pre"], mu_p, rstd_p, "p", work)
    mu_r, rstd_r = ln_stats(None, "r", work, x_dram=T["x_row"])
    hT_row = make_hT(T["xT_row"], mu_r, rstd_r, "r", work)

    Wxi = load_cast(T["W_in_xi"], DIM, DI, "Wxi")          # 2 x (128,512)
    Wz = load_cast(T["W_in_z"], DIM, DI, "Wz")
    Wxp = load_cast(T["W_xp"], DI, 144, "Wxp")             # 4 x (128,144)
    Wdt = load_cast(T["W_dtm"], DTR, DI, "Wdt")            # 1 x (16,512)
    ysel_b = load_cast(T["ysel"], 128, 64, "ysel", dest=wp)[0]

    smalls = {}
    for nm in ("b_in_xi", "b_in_z", "convw", "convb", "dtb", "Dp_c",
               "lnw_c", "lnb_c", "b_out_c", "b_fc1_c", "b_fc2_c"):
        tt = persist.tile(list(T[nm].shape), f32, tag=nm, name=nm)
        dma(tt[:], T[nm][:, :])
        smalls[nm] = tt

    # conv diag weight tiles: diag[j,tap] = diag(convw[:, j*9+tap])
    cdiag = []
    for j in range(4):
        row = []
        for tap in range(9):
            dg = work.tile([128, 128], bf16, tag=f"cdiag{j}_{tap}", name=f"cdiag{j}_{tap}")
            nc.vector.tensor_scalar_mul(dg[:], identb[:],
                                        smalls["convw"][:, j * 9 + tap:j * 9 + tap + 1])
            row.append(dg)
        cdiag.append(row)

    alog = work.tile([128, 256], f32, tag="stage", name="alog", bufs=2)
    dma(alog[:], T["a_logcols"][:, :])
    acols = wp.tile([128, 256], f32, tag="acols", name="acols")
    nc.scalar.activation(acols[:], alog[:], AF.Exp)
    dpdiag = []
    for jj in range(4):
        dpd = wp.tile([128, 64], bf16, tag=f"dpd{jj}", name=f"dpd{jj}")
        nc.vector.tensor_scalar_mul(dpd[:], ysel_b[:], smalls["Dp_c"][:, jj:jj + 1])
        dpdiag.append(dpd)

    gb_out = persist.tile([128, 2], f32, tag="gb_out", name="gb_out")
    nc.vector.tensor_tensor(gb_out[:], g_msa[:], smalls["b_out_c"][:], MUL)
    gb_fc2 = persist.tile([128, 2], f32, tag="gb_fc2", name="gb_fc2")
    nc.vector.tensor_tensor(gb_fc2[:], g_mlp[:], smalls["b_fc2_c"][:], MUL)

    # ---------------- S3: xi projection into padded conv planes -----------
    # pad[j]: (128, PADW) bf16, image interior at [1+h, 1+w] (34-wide rows)
    pads = []
    for j in range(4):
        pad = work.tile([128, PADW], bf16, tag=f"pad{j}", name=f"pad{j}")
        nc.gpsimd.memset(pad[:], 0.0)
        pads.append(pad)

    for j in range(4):
        ppm = pp.tile([128, L], f32, tag="projp", name=f"pp_xiT{j}", bufs=2)
        for kk in range(2):
            for th in range(2):
                nc.tensor.matmul(
                    ppm[:, th * 512:(th + 1) * 512],
                    Wxi[kk][:, j * 128:(j + 1) * 128],
                    hT_pre[kk][:, th * 512:(th + 1) * 512],
                    start=(kk == 0), stop=(kk == 1))
        pad3 = pads[j][:, :34 * 34].rearrange("p (H W) -> p H W", H=34, W=34)
        nc.scalar.activation(pad3[:, 1:33, 1:33],
                             ppm[:, :].rearrange("p (h w) -> p h w", h=32, w=32),
                             AF.Identity, bias=smalls["b_in_xi"][:, j:j + 1])

    # z projection (row domain) -> spill to DRAM for the post phase
    for j in range(4):
        ppz = pp.tile([128, L], f32, tag="projp", name=f"pp_zTs{j}", bufs=2)
        for kk in range(2):
            for th in range(2):
                nc.tensor.matmul(
                    ppz[:, th * 512:(th + 1) * 512],
                    Wz[kk][:, j * 128:(j + 1) * 128],
                    hT_row[kk][:, th * 512:(th + 1) * 512],
                    start=(kk == 0), stop=(kk == 1))
        zt = work.tile([128, L], bf16, tag="zTs", name=f"zTs{j}", bufs=2)
        nc.scalar.activation(zt[:], ppz[:], AF.Identity, bias=smalls["b_in_z"][:, j:j + 1])
        dma(T["z_spill"][j * 128:(j + 1) * 128, :], zt[:])

    # ---------------- S4: depthwise conv 3x3 via PE diag matmuls ----------
    xsT = []
    HHALF = 16
    HFREE = HHALF * 34
    for j in range(4):
        xs = wp.tile([128, L], bf16, tag=f"xsT{j}", name=f"xsT{j}")
        for hh in range(2):
            base = hh * HHALF * 34
            cvp = pp.tile([128, HFREE], f32, tag="convp", name=f"convp{j}_{hh}", bufs=2)
            for c0 in range(0, HFREE, 512):
                c1 = min(c0 + 512, HFREE)
                for tap in range(9):
                    dy, dx = tap // 3, tap % 3
                    off = base + dy * 34 + dx
                    nc.tensor.matmul(
                        cvp[:, c0:c1], cdiag[j][tap][:],
                        pads[j][:, off + c0:off + c1],
                        start=(tap == 0), stop=(tap == 8))
            nc.scalar.activation(
                xs[:, hh * HHALF * 32:(hh + 1) * HHALF * 32]
                  .rearrange("p (h w) -> p h w", h=HHALF, w=32),
                cvp[:, :].rearrange("p (h w) -> p h w", h=HHALF, w=34)[:, :, 0:32],
                AF.Silu, bias=smalls["convb"][:, j:j + 1])
        xsT.append(xs)

    # ---------------- S5: x_dbl = [B; C] and dt_r -------------------------
    bc_t = wp.tile([128, L], bf16, tag="bc_t", name="bc_t")
    ppbc = pp.tile([128, L], f32, tag="projp", name="ppbc", bufs=2)
    for kk in range(4):
        for th in range(2):
            nc.tensor.matmul(ppbc[:, th * 512:(th + 1) * 512],
                             Wxp[kk][:, 0:128],
                             xsT[kk][:, th * 512:(th + 1) * 512],
                             start=(kk == 0), stop=(kk == 3))
    nc.scalar.copy(bc_t[:], ppbc[:])
    dtr_t = work.tile([16, L], bf16, tag="dtr_t", name="dtr_t")
    ppdtr = pp.tile([16, L], f32, tag="projp", name="ppdtr", bufs=2)
    for kk in range(4):
        for th in range(2):
            nc.tensor.matmul(ppdtr[:, th * 512:(th + 1) * 512],
                             Wxp[kk][:, 128:144],
                             xsT[kk][:, th * 512:(th + 1) * 512],
                             start=(kk == 0), stop=(kk == 3))
    nc.scalar.copy(dtr_t[:], ppdtr[:])

    # ---------------- S6: dt, -dt broadcast, w broadcast ------------------
    wbc, ndtbc = [], []
    bcast_srcs = []
    for j in range(4):
        ppd = pp.tile([128, L], f32, tag="projp", name=f"ppdt{j}", bufs=2)
        for th in range(2):
            nc.tensor.matmul(ppd[:, th * 512:(th + 1) * 512],
                             Wdt[0][:, j * 128:(j + 1) * 128],
                             dtr_t[:, th * 512:(th + 1) * 512],
                             start=True, stop=True)
        spx = work.tile([128, L], f32, tag="spx", name=f"spx{j}", bufs=2)
        nc.scalar.activation(spx[:], ppd[:], AF.Exp, bias=smalls["dtb"][:, j:j + 1])
        dt_b = work.tile([128, L], bf16, tag="dtT", name=f"dtT{j}", bufs=2)
        nc.scalar.activation(dt_b[:], spx[:], AF.Ln, bias=1.0)
        ndt = wp.tile([128, L], bf16, tag="ndtT", name=f"ndtT{j}", bufs=2)
        nc.vector.tensor_scalar_mul(ndt[:], dt_b[:], -1.0)
        w_b = wp.tile([128, L], bf16, tag="wTtmp", name=f"wT{j}", bufs=2)
        nc.vector.tensor_tensor(w_b[:], dt_b[:], xsT[j][:], MUL)
        for par in range(2):
            g = 2 * j + par
            wb = wp.tile([128, L], bf16, tag=f"wbc{g}", name=f"wbc{g}")
            wbc.append(wb)
            nb = wp.tile([128, L], bf16, tag=f"ndtbc{g}", name=f"ndtbc{g}")
            ndtbc.append(nb)
            bcast_srcs.append((wb, w_b, nb, ndt, par))

    pre_ps.close()
    workstack.close()

    # ---------------- post-phase pools (opened early: tiles live across) --
    post = ExitStack()
    pf = post.enter_context(tc.tile_pool(name="postf", bufs=1))

    ones_f = pf.tile([128, 1], f32, tag="ones_f", name="ones_f")
    nc.gpsimd.memset(ones_f[:], 1.0)
    ones_b = pf.tile([128, 1], bf16, tag="ones_b", name="ones_b")
    nc.gpsimd.memset(ones_b[:], 1.0)

    siluz = []
    for j in range(4):
        zrl = pf.tile([128, L], bf16, tag="zrl", name=f"zrl{j}", bufs=2)
        dma(zrl[:], T["z_spill"][j * 128:(j + 1) * 128, :])
        sz = pf.tile([128, L], bf16, tag="siluz", name=f"siluz{j}", bufs=4)
        nc.scalar.activation(sz[:], zrl[:], AF.Silu)
        siluz.append(sz)

    # ---------------- S7: scan (g-major) + chunked AllGather + combine ----
    scan_st = ExitStack()
    yps = scan_st.enter_context(tc.tile_pool(name="yps", bufs=1, space="PSUM"))
    spool = scan_st.enter_context(tc.tile_pool(name="spool", bufs=2))
    bpool = scan_st.enter_context(tc.tile_pool(name="bpool", bufs=1))

    ssum = yps.tile([128, 8], f32, tag="ssum", name="ssum", bufs=1)
    ssq = yps.tile([128, 8], f32, tag="ssq", name="ssq", bufs=1)
    y_row = [None] * 4
    sq_y = [None] * 4
    yrps = [None] * 4

    # staggered post-weight loads: (dram, r0, r, h0, h1, dest_tile) pairs
    wload_specs = []
    for nm, dram, rows, cols in (("Wout", T["W_out"], DI, DIM),
                                 ("Wfc1", T["W_fc1"], DIM, HID),
                                 ("Wfc2", T["W_fc2"], HID, DIM)):
        n = (rows + 127) // 128
        tiles = []
        for jj in range(n):
            tb = pf.tile([128, cols], bf16, tag=f"{nm}_b{jj}", name=f"{nm}_b{jj}")
            tiles.append(tb)
            for h0 in range(0, cols, 512):
                wload_specs.append((dram, jj * 128, h0, min(h0 + 512, cols), tb))
        if nm == "Wout":
            Wout = tiles
        elif nm == "Wfc1":
            Wfc1 = tiles
        else:
            Wfc2 = tiles
    wload_pending = []
    wload_i = [0]

    def emit_wloads(ndma):
        for tf, tb, h0, h1 in wload_pending:
            nc.scalar.copy(tb[:, h0:h1], tf[:, :h1 - h0])
        wload_pending.clear()
        for _ in range(ndma):
            if wload_i[0] >= len(wload_specs):
                return
            dram, r0, h0, h1, tb = wload_specs[wload_i[0]]
            wload_i[0] += 1
            tf = pf.tile([128, 512], f32, tag="pstage", name=f"pst_w{wload_i[0]}", bufs=2)
            dma(tf[:, :h1 - h0], dram[r0:r0 + 128, h0:h1])
            wload_pending.append((tf, tb, h0, h1))

    def emit_combine(g, anchor=None, mk=None):
        j, par = g // 2, g % 2
        if par == 0:
            y_row[j] = pf.tile([128, L], bf16, tag="yrow", name=f"yrow{j}", bufs=4)
            sq_y[j] = pf.tile([128, L], bf16, tag="sqy", name=f"sq_y{j}", bufs=2)
        yr, sq = y_row[j], sq_y[j]
        r0, r1 = par * 64, par * 64 + 64
        ysk = []
        for k in range(4):
            t16 = pf.tile([64, L], bf16, tag="ysk", name=f"ysk{g}_{k}", bufs=4)
            ld = dma(t16[:], T[f"ys_g{g}"][k, :, :])
            if anchor is not None:
                _add_dep_helper(ld.ins, anchor.ins, sync=True,
                                reason="defer gather read past next group")
            ysk.append(t16)
        yrp = yrps[j]
        views = [
            ysk[0][:, :],
            (ysk[1][:, :].rearrange("p (w h) -> p w h", w=32, h=32)
                         .rearrange("p w h -> p h w")),
            ysk[2][:, ::-1],
            (ysk[3][:, ::-1].rearrange("p (w h) -> p w h", w=32, h=32)
                            .rearrange("p w h -> p h w")),
        ]
        for th in range(2):
            for k in range(4):
                v = views[k]
                if len(v.shape) == 3:
                    vth = v[:, th * 16:(th + 1) * 16, :]
                else:
                    vth = v[:, th * 512:(th + 1) * 512]
                nc.tensor.matmul(
                    yrp[r0:r1, th * 512:(th + 1) * 512],
                    identb[0:64, 0:64], vth,
                    start=(k == 0), stop=(k == 3))
        if mk is not None:
            nc.scalar.activation(yr[r0:r1, :], yrp[r0:r1, :], AF.Identity,
                                 bias=mk[r0:r1, 0:1])
            nc.scalar.activation(sq[r0:r1, :], yrp[r0:r1, :], AF.Square,
                                 bias=mk[r0:r1, 0:1])
        else:
            nc.scalar.copy(yr[r0:r1, :], yrp[r0:r1, :])
            nc.scalar.activation(sq[r0:r1, :], yrp[r0:r1, :], AF.Square)
        for tb in range(8):
            nc.tensor.matmul(ssum[:, tb:tb + 1],
                             yr[r0:r1, tb * 128:(tb + 1) * 128],
                             ones_b[r0:r1, :],
                             start=(g == 0), stop=(g == 7))
            nc.tensor.matmul(ssq[:, tb:tb + 1],
                             sq[r0:r1, tb * 128:(tb + 1) * 128],
                             ones_b[r0:r1, :],
                             start=(g == 0), stop=(g == 7))

    it = 0
    pending_g = []
    ys16_insts = [None] * 8
    mk_tiles = [None] * 8
    from concourse.bass import _add_dep_helper

    def emit_bcast(g):
        wb, w_b, nb, ndt, par = bcast_srcs[g]
        wsrc = w_b[par * 64:par * 64 + 64, :]
        dma(wb[0:64, :], wsrc)
        dma(wb[64:128, :], wsrc)
        nsrc = ndt[par * 64:par * 64 + 64, :]
        dma(nb[0:64, :], nsrc)
        dma(nb[64:128, :], nsrc)
    for g in range(2):
        emit_bcast(g)

    def scan_it(j, par, ch, il, ypt, Bb, Cb):
        nonlocal_it = scan_it.it
        g = 2 * j + par
        i = ch * CHUNK + il
        dA = spool.tile([128, L], bf16, tag="dA", name=f"dA{nonlocal_it}", bufs=3)
        nc.scalar.activation(dA[:], ndtbc[g][:], AF.Exp,
                             scale=acols[:, g * 32 + i:g * 32 + i + 1])
        if ch == 1 and il == 0:
            mkt = spool.tile([128, 1], bf16, tag="mk", name=f"mk{g}", bufs=8)
            nc.scalar.activation(mkt[:, 0:1], dA[:, 0:1], AF.Identity, scale=0.0)
            mk_tiles[g] = mkt
        xin = spool.tile([128, L], bf16, tag="xin", name=f"xin{nonlocal_it}", bufs=3)
        xeng = nc.vector if (nonlocal_it % 13 in (0, 6)) else nc.gpsimd
        xeng.tensor_tensor(xin[:], wbc[g][:], Bb[il][:], MUL)
        h = spool.tile([128, L], bf16, tag="h", name=f"h{nonlocal_it}", bufs=3)
        nc.vector.tensor_tensor_scan(h[:], dA[:], xin[:], 0.0, MUL, ADD)
        yc = spool.tile([128, L], bf16, tag="yc", name=f"yc{nonlocal_it}", bufs=3)
        nc.vector.tensor_tensor(yc[:], h[:], Cb[il][:], MUL)
        for th in range(2):
            nc.tensor.matmul(
                ypt[par * 64:par * 64 + 64, th * 512:(th + 1) * 512],
                ysel_b[:], yc[:, th * 512:(th + 1) * 512],
                start=(ch == 0 and il == 0), stop=False)
        scan_it.it += 1
    scan_it.it = 0

    def load_bc(j, ch, sfx=""):
        Bb, Cb = [], []
        for il in range(CHUNK):
            i = ch * CHUNK + il
            bbt = bpool.tile([128, L], bf16, tag="Bb", name=f"Bb{sfx}{j}_{ch}_{il}",
                             bufs=2 * CHUNK - 2)
            dma(bbt[:], bc_t[2 * i:2 * i + 2, :]
                .partition_broadcast(64).rearrange("d n f -> n d f"))
            Bb.append(bbt)
            cbt = bpool.tile([128, L], bf16, tag="Cb", name=f"Cb{sfx}{j}_{ch}_{il}",
                             bufs=2 * CHUNK - 2)
            dma(cbt[:], bc_t[64 + 2 * i:64 + 2 * i + 2, :]
                .partition_broadcast(64).rearrange("d n f -> n d f"))
            Cb.append(cbt)
        return Bb, Cb

    def emit_tail(j, par, ypt):
        g = 2 * j + par
        r0, r1 = par * 64, par * 64 + 64
        for th in range(2):
            nc.tensor.matmul(
                ypt[r0:r1, th * 512:(th + 1) * 512],
                dpdiag[j][r0:r1, :], xsT[j][r0:r1, th * 512:(th + 1) * 512],
                start=False, stop=(th == 1))
        ysb16 = spool.tile([64, L], bf16, tag="ys16", name=f"ys16_{g}", bufs=1)
        ys16_insts[g] = nc.scalar.copy(ysb16[:], ypt[r0:r1, :])
        dma(T["ys_lb"][g * 64:(g + 1) * 64, :], ysb16[:])
        nc.gpsimd.collective_compute(
            "AllGather", BYP,
            replica_groups=[[0, 1, 2, 3], [4, 5, 6, 7]],
            ins=[T["ys_lb"][g * 64:(g + 1) * 64, :]],
            outs=[T[f"ys_g{g}"][:, :, :]],
        )
        pending_g.append(g)

    for j in range(3):
        ypt = yps.tile([128, L], f32, tag="ypt", name=f"ypt{j}", bufs=2)
        yrps[j] = yps.tile([128, L], f32, tag="yrp", name=f"yrp{j}", bufs=1)
        if j > 0:
            emit_bcast(2 * j)
            emit_bcast(2 * j + 1)
        for par in range(2):
            for ch in range(NCHUNK):
                if ch in (2, 5) and len(pending_g) > 1:
                    gq = pending_g.pop(0)
                    emit_combine(gq, anchor=ys16_insts[gq + 1],
                                 mk=mk_tiles[gq + 2] if gq + 2 < 8 else None)
                if ch in (3, 6):
                    emit_wloads(1)
                Bb, Cb = load_bc(j, ch, sfx=f"q{par}_")
                for il in range(CHUNK):
                    scan_it(j, par, ch, il, ypt, Bb, Cb)
            emit_tail(j, par, ypt)

    # last j: g-major so the two final gathers stagger
    emit_bcast(6)
    emit_bcast(7)
    j = 3
    ypt = yps.tile([128, L], f32, tag="ypt", name=f"ypt{j}", bufs=2)
    yrps[j] = yps.tile([128, L], f32, tag="yrp", name=f"yrp{j}", bufs=1)
    for par in range(2):
        for ch in range(NCHUNK):
            if ch in (2, 5) and len(pending_g) > 1:
                gq = pending_g.pop(0)
                emit_combine(gq, anchor=ys16_insts[gq + 1],
                             mk=mk_tiles[gq + 2] if gq + 2 < 8 else None)
            if ch in (3, 6):
                emit_wloads(1)
            Bb, Cb = load_bc(j, ch, sfx=f"p{par}_")
            for il in range(CHUNK):
                scan_it(j, par, ch, il, ypt, Bb, Cb)
        emit_tail(j, par, ypt)

    while pending_g:
        emit_wloads(0)
        gq = pending_g.pop(0)
        emit_combine(gq, anchor=ys16_insts[gq + 1] if gq + 1 < 8 else None,
                     mk=mk_tiles[gq + 2] if gq + 2 < 8 else None)
    emit_wloads(16)
    emit_wloads(0)

    # ---------------- S12: LN over d + ln_w/ln_b, gate --------------------
    def stat_cols(ssum_t, ssq_t, dim, name):
        mu = pf.tile([128, 8], f32, tag="pmu", name=f"pmu_{name}", bufs=2)
        nc.vector.tensor_scalar_mul(mu[:], ssum_t[:], 1.0 / dim)
        mu2 = pf.tile([128, 8], f32, tag="pmu2", name=f"pmu2_{name}", bufs=2)
        nc.vector.tensor_tensor(mu2[:], mu[:], mu[:], MUL)
        var = pf.tile([128, 8], f32, tag="pvar", name=f"pvar_{name}", bufs=2)
        nc.vector.scalar_tensor_tensor(var[:], ssq_t[:], 1.0 / dim, mu2[:], MUL, SUB)
        std = pf.tile([128, 8], f32, tag="pstd", name=f"pstd_{name}", bufs=2)
        nc.scalar.activation(std[:], var[:], AF.Sqrt, bias=eps_col[:, 0:1])
        rstd = pf.tile([128, 8], f32, tag="prstd", name=f"prstd_{name}", bufs=2)
        nc.vector.reciprocal(rstd[:], std[:])
        return mu, rstd

    mu_y, rstd_y = stat_cols(ssum, ssq, DI, "y")

    scan_st.close()

    ppost = post.enter_context(tc.tile_pool(name="ps_post", bufs=2, space="PSUM"))
    ymu_bc = bcast_cols(mu_y, "Pym", pf, "bcA", ppost, "php", dt_out=bf16)
    yrstd_bc = bcast_cols(rstd_y, "Pyr", pf, "bcB", ppost, "php", dt_out=bf16)

    gated = []
    for j in range(4):
        t1 = pf.tile([128, L], bf16, tag="psc", name=f"lny1_{j}", bufs=2)
        nc.vector.tensor_tensor(t1[:], y_row[j][:], ymu_bc[:], SUB)
        t2 = pf.tile([128, L], bf16, tag="psc", name=f"lny2_{j}", bufs=2)
        nc.vector.tensor_tensor(t2[:], t1[:], yrstd_bc[:], MUL)
        t3 = pf.tile([128, L], bf16, tag="psc", name=f"lny3_{j}", bufs=2)
        nc.scalar.activation(t3[:], t2[:], AF.Identity,
                             bias=smalls["lnb_c"][:, j:j + 1],
                             scale=smalls["lnw_c"][:, j:j + 1])
        gt = pf.tile([128, L], bf16, tag=f"gated{j}", name=f"gated{j}")
        nc.vector.tensor_tensor(gt[:], t3[:], siluz[j][:], MUL)
        gated.append(gt)

    # ---------------- S14: hy, residual -----------------------------------
    xTrs = []
    for cc in range(2):
        xt = pf.tile([128, L], f32, tag="xTrl", name=f"xTr{cc}", bufs=2)
        dma(xt[:], T["xT_row"][cc * 128:(cc + 1) * 128, :])
        xTrs.append(xt)
    x2T = []
    for cc in range(2):
        php = ppost.tile([128, L], f32, tag="php", name=f"php{cc}", bufs=2)
        for kk in range(4):
            for th in range(2):
                nc.tensor.matmul(php[:, th * 512:(th + 1) * 512],
                                 Wout[kk][:, cc * 128:(cc + 1) * 128],
                                 gated[kk][:, th * 512:(th + 1) * 512],
                                 start=(kk == 0), stop=(kk == 3))
        t1 = pf.tile([128, L], f32, tag="pscf", name=f"hyg{cc}", bufs=2)
        nc.scalar.activation(t1[:], php[:], AF.Identity,
                             bias=gb_out[:, cc:cc + 1], scale=g_msa[:, cc:cc + 1])
        x2 = pf.tile([128, L], bf16, tag=f"x2T{cc}", name=f"x2T{cc}")
        with nc.allow_low_precision(reason="x2 residual in bf16; tol 2e-2"):
            nc.vector.tensor_tensor(x2[:], t1[:], xTrs[cc][:], ADD)
        x2T.append(x2)

    # ---------------- S15: LN2 + modulate ---------------------------------
    x2sum = ppost.tile([128, 8], f32, tag="pfc2p", name="x2sum", bufs=2)
    x2sq = ppost.tile([128, 8], f32, tag="pfc2p", name="x2sq", bufs=2)
    sqx = []
    for ccj in range(2):
        sq = pf.tile([128, L], bf16, tag="sqt", name=f"sq_x2{ccj}", bufs=2)
        nc.scalar.activation(sq[:], x2T[ccj][:], AF.Square)
        sqx.append(sq)
    for tb in range(8):
        for ccj in range(2):
            nc.tensor.matmul(x2sum[:, tb:tb + 1],
                             x2T[ccj][:, tb * 128:(tb + 1) * 128], ones_b[:],
                             start=(ccj == 0), stop=(ccj == 1))
            nc.tensor.matmul(x2sq[:, tb:tb + 1],
                             sqx[ccj][:, tb * 128:(tb + 1) * 128], ones_b[:],
                             start=(ccj == 0), stop=(ccj == 1))
    mu_x2, rstd_x2 = stat_cols(x2sum, x2sq, DIM, "x2")
    x2mu_bc = bcast_cols(mu_x2, "Px2m", pf, "bcA", ppost, "php", dt_out=bf16)
    x2rstd_bc = bcast_cols(rstd_x2, "Px2r", pf, "bcB", ppost, "php", dt_out=bf16)
    mT = []
    for cc in range(2):
        t1 = pf.tile([128, L], bf16, tag="psc", name=f"m1_{cc}", bufs=2)
        nc.vector.tensor_tensor(t1[:], x2T[cc][:], x2mu_bc[:], SUB)
        t2 = pf.tile([128, L], bf16, tag="psc", name=f"m2_{cc}", bufs=2)
        nc.vector.tensor_tensor(t2[:], t1[:], x2rstd_bc[:], MUL)
        mb = pf.tile([128, L], bf16, tag=f"mT{cc}", name=f"mT{cc}")
        nc.scalar.activation(mb[:], t2[:], AF.Identity,
                             bias=sh_mlp[:, cc:cc + 1], scale=s1_mlp[:, cc:cc + 1])
        mT.append(mb)

    # ---------------- S16: MLP + final residual ---------------------------
    pfc2ps = [ppost.tile([128, L], f32, tag="pfc2p", name=f"pfc2_{cc}", bufs=2)
              for cc in range(2)]
    for kk in range(8):
        pfc = ppost.tile([128, L], f32, tag="php", name=f"pfc1_{kk}", bufs=2)
        for kk2 in range(2):
            for th in range(2):
                nc.tensor.matmul(pfc[:, th * 512:(th + 1) * 512],
                                 Wfc1[kk2][:, kk * 128:(kk + 1) * 128],
                                 mT[kk2][:, th * 512:(th + 1) * 512],
                                 start=(kk2 == 0), stop=(kk2 == 1))
        gl = pf.tile([128, L], bf16, tag="gelu", name=f"gelu{kk}", bufs=2)
        nc.scalar.activation(gl[:], pfc[:], AF.Gelu_apprx_tanh, bias=smalls["b_fc1_c"][:, kk:kk + 1])
        for cc in range(2):
            for th in range(2):
                nc.tensor.matmul(pfc2ps[cc][:, th * 512:(th + 1) * 512],
                                 Wfc2[kk][:, cc * 128:(cc + 1) * 128],
                                 gl[:, th * 512:(th + 1) * 512],
                                 start=(kk == 0), stop=(kk == 7))

    for cc in range(2):
        t1 = pf.tile([128, L], f32, tag="pscf", name=f"mlpg{cc}", bufs=2)
        nc.scalar.activation(t1[:], pfc2ps[cc][:], AF.Identity,
                             bias=gb_fc2[:, cc:cc + 1], scale=g_mlp[:, cc:cc + 1])
        o = pf.tile([128, L], f32, tag="outTt", name=f"outT{cc}", bufs=1)
        nc.vector.tensor_tensor(o[:], t1[:], x2T[cc][:], ADD)
        dma(T["outT"][cc * 128:(cc + 1) * 128, :], o[:])

    post.close()
    wstack.close()
    perstack.close()


# ---------------------------------------------------------------------------
# Host side
_PROGRAM = None


def _get_program():
    global _PROGRAM
    if _PROGRAM is None:
        _PROGRAM = build_program()
    return _PROGRAM


def _q_img(x, k):
    img = x.reshape(Hs, Ws, -1)
    if k == 0:
        out = img
    elif k == 1:
        out = img.transpose(1, 0, 2)
    elif k == 2:
        out = img[::-1, ::-1]
    else:
        out = img.transpose(1, 0, 2)[::-1, ::-1]
    return np.ascontiguousarray(out.reshape(L, -1))


def _conv_w_q(w, k):
    if k == 0:
        return w
    if k == 1:
        return np.ascontiguousarray(w.transpose(1, 0, 2))
    if k == 2:
        return np.ascontiguousarray(w[::-1, ::-1])
    return np.ascontiguousarray(w.transpose(1, 0, 2)[::-1, ::-1])


def _col128(v, ncols):
    return np.ascontiguousarray(v.reshape(ncols, 128).T)


def prep_inputs(inputs):
    inp = {k: np.asarray(v, dtype=np.float32) for k, v in inputs.items()}
    x, c = inp["x"], inp["c"]

    shared = {}
    shared["W_ada"] = inp["W_ada"]
    shared["b_ada"] = inp["b_ada"].reshape(1, 6 * DIM)
    W_in = inp["W_in"]
    shared["W_in_xi"] = np.ascontiguousarray(W_in[:, :DI])
    shared["W_in_z"] = np.ascontiguousarray(W_in[:, DI:])
    b_in = inp["b_in"]
    shared["b_in_xi"] = _col128(b_in[:DI], 4)
    shared["b_in_z"] = _col128(b_in[DI:], 4)
    shared["lnw_c"] = _col128(inp["ln_w"], 4)
    shared["lnb_c"] = _col128(inp["ln_b"], 4)
    shared["W_out"] = inp["W_out"]
    shared["b_out_c"] = _col128(inp["b_out"], 2)
    shared["W_fc1"] = inp["W_fc1"]
    shared["b_fc1_c"] = _col128(inp["b_fc1"], 8)
    shared["W_fc2"] = inp["W_fc2"]
    shared["b_fc2_c"] = _col128(inp["b_fc2"], 2)
    shared["ident"] = np.eye(128, dtype=np.float32)
    p = np.arange(128)
    ys = np.zeros((128, 64), np.float32)
    ys[p, p % 64] = 1.0
    shared["ysel"] = ys

    in_maps = []
    for core in range(8):
        b, k = core // 4, core % 4
        m = dict(shared)
        xb = x[b]
        xpre = _q_img(xb, k)
        m["x_pre"] = xpre
        m["xT_pre"] = np.ascontiguousarray(xpre.T)
        m["x_row"] = np.ascontiguousarray(xb)
        m["xT_row"] = np.ascontiguousarray(xb.T)
        m["c_vec"] = c[b].reshape(1, DIM)

        cw = _conv_w_q(inp["conv_w"].reshape(3, 3, DI), k)
        cwt = cw.reshape(9, 4, 128)                       # [tap, dtile, p]
        m["convw"] = np.ascontiguousarray(cwt.transpose(2, 1, 0).reshape(128, 36))
        m["convb"] = _col128(inp["conv_b"], 4)

        Wxp = inp["W_xproj"][k]                           # (DI, 144) cols [dtr,B,C]
        m["W_xp"] = np.ascontiguousarray(
            np.concatenate([Wxp[:, DTR:DTR + DS], Wxp[:, DTR + DS:], Wxp[:, :DTR]],
                           axis=1))
        m["W_dtm"] = np.ascontiguousarray(inp["W_dt"][k])
        m["dtb"] = _col128(inp["dt_bias"][k], 4)
        m["Dp_c"] = _col128(inp["Dp"][k], 4)

        alog = inp["A_log"][k]                            # (DI, DS)
        acols = np.zeros((128, 256), np.float32)
        for g in range(8):
            for i in range(NPAIRS):
                acols[:, g * 32 + i] = alog[g * 64 + (p % 64), 2 * i + (p // 64)]
        m["a_logcols"] = acols
        in_maps.append(m)
    return in_maps


def kernel(**inputs):
    nc = _get_program()
    in_maps = prep_inputs(inputs)
    res = run_bass_kernel_spmd(nc, in_maps, list(range(8)))
    out = np.zeros((B, L, DIM), np.float32)
    for b in range(B):
        out[b] = res.results[4 * b]["outT"].T
    return out


# revision 87
# speedup vs baseline: 1.2139x; 1.0099x over previous
"""DiM block (Mamba-style selective-scan transformer block) on 8 TRN2 cores.

Sharding: core i handles (b = i//4, k = i%4) — one batch sample and one of
the 4 scan directions. The spatial permutation q_k (identity / transpose /
flip / anti-transpose) is pushed onto host-prepared inputs (x pre-permuted,
conv taps transformed) so ONE SPMD program serves all 8 cores. The per-sample
combine over k happens on-device via per-d-group AllGathers within each
4-core group (chunked so they overlap the scan), followed by compile-time
(uniform) re-orderings done as identity-matmuls with permuted moving APs.

Scan layout: partitions = (n-pair:2 outer, d:64 inner), free = t.
Per (d-group g, n-pair i): ACT computes dA = exp(A*dt) from a pre-broadcast
-dt tile, DVE does the w*B and h*C multiplies (bf16 2x mode), the hardware
scan runs mostly on GPSIMD (1/8 on DVE to balance), PE reduces the n-pair
into a per-d-group PSUM y accumulator that lives across the whole group.
"""
import json
import sys

sys.path.insert(0, "/opt/trn_rl_repo")

import numpy as np
import concourse.bass as bass
import concourse.mybir as mybir
import concourse.tile as tile
from concourse.bass_utils import run_bass_kernel_spmd

# ---------------------------------------------------------------------------
# Workaround: this walrus build rejects instructions carrying >1 embedded
# sem-wait. Split extra waits onto same-engine NoOps at BIR serialization.
_MAXW = 1
_wsplit_counter = [0]


def _split_multi_waits(bir: dict) -> dict:
    for fn in bir.get("functions", []):
        for bb in fn.get("blocks", []):
            insts = bb.get("instructions", [])
            if not any(
                len((i.get("sync_info") or {}).get("on_wait") or []) > _MAXW
                for i in insts
            ):
                continue
            out = []
            for inst in insts:
                si = inst.get("sync_info")
                waits = (si or {}).get("on_wait") or []
                if len(waits) > _MAXW and inst.get("engine"):
                    for w in waits[:-_MAXW]:
                        _wsplit_counter[0] += 1
                        out.append({
                            "debug": inst.get("debug", 0),
                            "engine": inst["engine"],
                            "ins": [], "outs": [],
                            "name": f"I-wsplit-{_wsplit_counter[0]}",
                            "opcode": "NoOp",
                            "sync_info": {"on_update": [], "on_wait": [w]},
                        })
                    si["on_wait"] = waits[-_MAXW:]
                out.append(inst)
            bb["instructions"] = out
    return bir


_orig_to_json_bytes = bass.Bass.to_json_bytes


def _patched_to_json_bytes(self) -> bytes:
    j = json.loads(_orig_to_json_bytes(self))
    _split_multi_waits(j)
    return json.dumps(j).encode()


bass.Bass.to_json_bytes = _patched_to_json_bytes

# ---------------------------------------------------------------------------
B, Hs, Ws, DIM = 2, 32, 32, 256
L = Hs * Ws
DI = 2 * DIM
DS = 64
DTR = DIM // 16
K = 4
HID = 4 * DIM

f32 = mybir.dt.float32
bf16 = mybir.dt.bfloat16
MUL = mybir.AluOpType.mult
ADD = mybir.AluOpType.add
SUB = mybir.AluOpType.subtract
BYP = mybir.AluOpType.bypass
AF = mybir.ActivationFunctionType
AX = mybir.AxisListType

EPS = 1e-6
NPAIRS = DS // 2          # 32 n-pairs
CHUNK = 4                 # n-pairs per chunk
NCHUNK = NPAIRS // CHUNK  # 8
PADW = 34 * 34 + 8        # padded conv plane + slack for shifted windows
CFREE = 1088              # conv output plane (32 rows x 34 cols)


def build_program(dve_scan_mod=0, pe_combine=True):
    nc = bass.Bass()

    def din(name, shape, dt=f32):
        return nc.dram_tensor(name, list(shape), dt, kind="ExternalInput")

    T = {}
    T["x_pre"] = din("x_pre", (L, DIM))
    T["xT_pre"] = din("xT_pre", (DIM, L))
    T["x_row"] = din("x_row", (L, DIM))
    T["xT_row"] = din("xT_row", (DIM, L))
    T["c_vec"] = din("c_vec", (1, DIM))
    T["W_ada"] = din("W_ada", (DIM, 6 * DIM))
    T["b_ada"] = din("b_ada", (1, 6 * DIM))
    T["W_in_xi"] = din("W_in_xi", (DIM, DI))
    T["W_in_z"] = din("W_in_z", (DIM, DI))
    T["b_in_xi"] = din("b_in_xi", (128, 4))
    T["b_in_z"] = din("b_in_z", (128, 4))
    T["convw"] = din("convw", (128, 36))      # [d_part, dtile*9 + tap]
    T["convb"] = din("convb", (128, 4))
    T["W_xp"] = din("W_xp", (DI, 144))        # cols reordered [B(64), C(64), dtr(16)]
    T["W_dtm"] = din("W_dtm", (DTR, DI))
    T["dtb"] = din("dtb", (128, 4))
    T["a_logcols"] = din("a_logcols", (128, 256))  # [p, g*32+i] = A_log[k, g*64+p%64, 2i+p//64]
    T["ysel"] = din("ysel", (128, 64))        # [p, d] = (p%64 == d)
    T["Dp_c"] = din("Dp_c", (128, 4))
    T["lnw_c"] = din("lnw_c", (128, 4))
    T["lnb_c"] = din("lnb_c", (128, 4))
    T["W_out"] = din("W_out", (DI, DIM))
    T["b_out_c"] = din("b_out_c", (128, 2))
    T["W_fc1"] = din("W_fc1", (DIM, HID))
    T["b_fc1_c"] = din("b_fc1_c", (128, 8))
    T["W_fc2"] = din("W_fc2", (HID, DIM))
    T["b_fc2_c"] = din("b_fc2_c", (128, 2))
    T["ident"] = din("ident", (128, 128))

    T["outT"] = nc.dram_tensor("outT", [DIM, L], f32, kind="ExternalOutput")
    T["ys_lb"] = nc.dram_tensor("ys_lb", [DI, L], bf16)
    for g in range(8):
        T[f"ys_g{g}"] = nc.dram_tensor(f"ys_g{g}", [4, 64, L], bf16)
    T["z_spill"] = nc.dram_tensor("z_spill", [DI, L], bf16)

    with tile.TileContext(nc) as tc:
        _build_body(nc, tc, T, dve_scan_mod, pe_combine)
    return nc


def _build_body(nc, tc, T, dve_scan_mod, pe_combine):
    from contextlib import ExitStack

    dma = nc.sync.dma_start

    perstack = ExitStack()
    persist = perstack.enter_context(tc.tile_pool(name="persist", bufs=1))
    wstack = ExitStack()
    wp = wstack.enter_context(tc.tile_pool(name="weights", bufs=1))
    workstack = ExitStack()
    work = workstack.enter_context(tc.tile_pool(name="work", bufs=1))
    pre_ps = ExitStack()
    pp = pre_ps.enter_context(tc.tile_pool(name="ps_pre", bufs=2, space="PSUM"))

    # ---------------- early x loads (these gate the pre-phase chain) ------
    xstat_tiles = {}
    tl = []
    for j in range(8):
        xt = work.tile([128, DIM], f32, tag="lnx", name=f"lnx_pre{j}", bufs=8)
        dma(xt[:], T["x_pre"][j * 128:(j + 1) * 128, :])
        tl.append(xt)
    xstat_tiles["x_pre"] = tl

    # ---------------- S0: load + cast weights (staged through 2 slots) ----
    def load_cast(dram, rows, cols, name, dest=None):
        dest = dest or work
        n = (rows + 127) // 128
        outs = []
        for j in range(n):
            r0 = j * 128
            r = min(128, rows - r0)
            tf = work.tile([r, cols], f32, tag="stage", name=f"st_{name}{j}", bufs=2)
            dma(tf[:], dram[r0:r0 + r, :])
            tb = dest.tile([r, cols], bf16, tag=f"{name}_b{j}", name=f"{name}_b{j}")
            nc.gpsimd.tensor_copy(tb[:], tf[:])
            outs.append(tb)
        return outs

    identf = persist.tile([128, 128], f32, tag="identf", name="identf")
    dma(identf[:], T["ident"][:, :])
    identb = persist.tile([128, 128], bf16, tag="identb", name="identb")
    nc.gpsimd.tensor_copy(identb[:], identf[:])

    Wada = []
    for j in range(2):
        tb = work.tile([128, 6 * DIM], bf16, tag=f"Wada_b{j}", name=f"Wada_b{j}")
        for hcol in range(2):
            tf = work.tile([128, 1024], f32, tag="stage", name=f"st_Wada{j}_{hcol}", bufs=2)
            dma(tf[:, :768], T["W_ada"][j * 128:(j + 1) * 128, hcol * 768:(hcol + 1) * 768])
            nc.vector.tensor_copy(tb[:, hcol * 768:(hcol + 1) * 768], tf[:, :768])
        Wada.append(tb)
    eps_col = persist.tile([128, 1], f32, tag="eps_col", name="eps_col")
    nc.gpsimd.memset(eps_col[:], EPS)
    # ---------------- S1: adaLN modulation vector -------------------------
    c_t = work.tile([1, DIM], f32, tag="c_t", name="c_t")
    dma(c_t[:], T["c_vec"][:, :])
    c_silu = work.tile([1, DIM], f32, tag="c_silu", name="c_silu")
    nc.scalar.activation(c_silu[:], c_t[:], AF.Silu)
    c_colp = pp.tile([128, 2], f32, tag="projp", name="c_colp", bufs=2)
    for half in range(2):
        nc.tensor.transpose(c_colp[:, half:half + 1],
                            c_silu[0:1, half * 128:(half + 1) * 128],
                            identf[0:1, 0:1])
    c_colb = work.tile([128, 2], bf16, tag="c_colb", name="c_colb")
    nc.scalar.copy(c_colb[:], c_colp[:])

    mod = work.tile([1, 6 * DIM], f32, tag="mod", name="mod")
    for fb in range(3):
        bada_s = work.tile([1, 512], f32, tag="bada_s", name=f"bada_s{fb}", bufs=2)
        dma(bada_s[:], T["b_ada"][0:1, fb * 512:(fb + 1) * 512])
        pmod = pp.tile([1, 512], f32, tag="projp", name=f"pmod{fb}", bufs=2)
        for kk in range(2):
            nc.tensor.matmul(pmod[:], c_colb[:, kk:kk + 1],
                             Wada[kk][:, fb * 512:(fb + 1) * 512],
                             start=(kk == 0), stop=(kk == 1))
        nc.vector.tensor_tensor(mod[:, fb * 512:(fb + 1) * 512], pmod[:],
                                bada_s[:], ADD)

    mcols = []
    for i6 in range(6):
        mcp = pp.tile([128, 2], f32, tag="projp", name=f"mcolp{i6}", bufs=2)
        for half in range(2):
            nc.tensor.transpose(
                mcp[:, half:half + 1],
                mod[0:1, i6 * 256 + half * 128:i6 * 256 + (half + 1) * 128],
                identf[0:1, 0:1])
        col = persist.tile([128, 2], f32, tag=f"mcol{i6}", name=f"mcol{i6}")
        nc.scalar.copy(col[:], mcp[:])
        mcols.append(col)
    sh_msa, sc_msa, g_msa, sh_mlp, sc_mlp, g_mlp = mcols
    s1_msa = persist.tile([128, 2], f32, tag="s1_msa", name="s1_msa")
    nc.scalar.activation(s1_msa[:], sc_msa[:], AF.Identity, bias=1.0)
    s1_mlp = persist.tile([128, 2], f32, tag="s1_mlp", name="s1_mlp")
    nc.scalar.activation(s1_mlp[:], sc_mlp[:], AF.Identity, bias=1.0)

    # ---------------- shared helpers -------------------------------------
    def ln_stats(xtiles, name, pool, x_dram=None):
        mu = pool.tile([128, 8], f32, tag=f"mu_{name}", name=f"mu_{name}")
        sq = pool.tile([128, 8], f32, tag=f"sq_{name}", name=f"sq_{name}")
        for j in range(8):
            if xtiles is None:
                xt = pool.tile([128, DIM], f32, tag="lnxr", name=f"lnxr_{name}{j}", bufs=2)
                dma(xt[:], x_dram[j * 128:(j + 1) * 128, :])
            else:
                xt = xtiles[j]
            nc.vector.tensor_reduce(mu[:, j:j + 1], xt[:], AX.X, ADD)
            scr = pool.tile([128, DIM], f32, tag="lnscr", name=f"lnscr_{name}{j}", bufs=2)
            nc.scalar.activation(scr[:], xt[:], AF.Square, accum_out=sq[:, j:j + 1])
        nc.vector.tensor_scalar_mul(mu[:], mu[:], 1.0 / DIM)
        mu2 = pool.tile([128, 8], f32, tag="ln_mu2", name=f"mu2_{name}")
        nc.vector.tensor_tensor(mu2[:], mu[:], mu[:], MUL)
        var = pool.tile([128, 8], f32, tag="ln_var", name=f"var_{name}")
        nc.vector.scalar_tensor_tensor(var[:], sq[:], 1.0 / DIM, mu2[:], MUL, SUB)
        std = pool.tile([128, 8], f32, tag="ln_std", name=f"std_{name}")
        nc.scalar.activation(std[:], var[:], AF.Sqrt, bias=eps_col[:, 0:1])
        rstd = pool.tile([128, 8], f32, tag=f"rstd_{name}", name=f"rstd_{name}")
        nc.vector.reciprocal(rstd[:], std[:])
        return mu, rstd

    def bcast_cols(stat, name, pool, tag, psum_pool, ptag, dt_out=f32):
        """(128,8) per-token stat -> (128,1024) all-partition broadcast tile."""
        statT_p = psum_pool.tile([8, 128], f32, tag=ptag, name=f"sTp_{name}", bufs=2)
        nc.tensor.transpose(statT_p[:], stat[:], identf[:])
        statT = pool.tile([8, 128], dt_out, tag="statT", name=f"sT_{name}", bufs=2)
        nc.scalar.copy(statT[:], statT_p[:])
        row2 = pool.tile([2, L], dt_out, tag=f"row2{dt_out}", name=f"r2_{name}", bufs=2)
        dma(row2[0:1, :], statT[:, :])
        dma(row2[1:2, :], statT[:, :])
        bc = pool.tile([128, L], dt_out, tag=tag, name=f"bc_{name}", bufs=1)
        dma(bc[:], row2[:, :].partition_broadcast(64).rearrange("n d f -> d n f"))
        return bc

    def make_hT(xT_dram, mu, rstd, name, pool):
        mu_bc = bcast_cols(mu, f"mu{name}", pool, "bcA", pp, "projp")
        rstd_bc = bcast_cols(rstd, f"rs{name}", pool, "bcB", pp, "projp")
        outs = []
        for cc in range(2):
            xt = pool.tile([128, L], f32, tag="hscr", name=f"hx_{name}{cc}", bufs=2)
            dma(xt[:], xT_dram[cc * 128:(cc + 1) * 128, :])
            t1 = pool.tile([128, L], f32, tag="hscr", name=f"hs1_{name}{cc}", bufs=2)
            nc.vector.tensor_tensor(t1[:], xt[:], mu_bc[:], SUB)
            t2 = pool.tile([128, L], f32, tag="hscr", name=f"hs2_{name}{cc}", bufs=2)
            nc.vector.tensor_tensor(t2[:], t1[:], rstd_bc[:], MUL)
            hb = pool.tile([128, L], bf16, tag=f"hTb_{name}{cc}", name=f"hTb_{name}{cc}")
            nc.scalar.activation(hb[:], t2[:], AF.Identity,
                                 bias=sh_msa[:, cc:cc + 1], scale=s1_msa[:, cc:cc + 1])
            outs.append(hb)
        return outs

    # ---------------- S2: LN1 + modulate (pre and row domains) ------------
    mu_p, rstd_p = ln_stats(xstat_tiles["x_pre"], "p", work)
    hT_pre = make_hT(T["xT_pre"], mu_p, rstd_p, "p", work)
    mu_r, rstd_r = ln_stats(None, "r", work, x_dram=T["x_row"])
    hT_row = make_hT(T["xT_row"], mu_r, rstd_r, "r", work)

    Wxi = load_cast(T["W_in_xi"], DIM, DI, "Wxi")          # 2 x (128,512)
    Wz = load_cast(T["W_in_z"], DIM, DI, "Wz")
    Wxp = load_cast(T["W_xp"], DI, 144, "Wxp")             # 4 x (128,144)
    Wdt = load_cast(T["W_dtm"], DTR, DI, "Wdt")            # 1 x (16,512)
    ysel_b = load_cast(T["ysel"], 128, 64, "ysel", dest=wp)[0]

    smalls = {}
    for nm in ("b_in_xi", "b_in_z", "convw", "convb", "dtb", "Dp_c",
               "lnw_c", "lnb_c", "b_out_c", "b_fc1_c", "b_fc2_c"):
        tt = persist.tile(list(T[nm].shape), f32, tag=nm, name=nm)
        dma(tt[:], T[nm][:, :])
        smalls[nm] = tt

    # conv diag weight tiles: diag[j,tap] = diag(convw[:, j*9+tap])
    cdiag = []
    for j in range(4):
        row = []
        for tap in range(9):
            dg = work.tile([128, 128], bf16, tag=f"cdiag{j}_{tap}", name=f"cdiag{j}_{tap}")
            nc.vector.tensor_scalar_mul(dg[:], identb[:],
                                        smalls["convw"][:, j * 9 + tap:j * 9 + tap + 1])
            row.append(dg)
        cdiag.append(row)

    alog = work.tile([128, 256], f32, tag="stage", name="alog", bufs=2)
    dma(alog[:], T["a_logcols"][:, :])
    acols = wp.tile([128, 256], f32, tag="acols", name="acols")
    nc.scalar.activation(acols[:], alog[:], AF.Exp)
    dpdiag = []
    for jj in range(4):
        dpd = wp.tile([128, 64], bf16, tag=f"dpd{jj}", name=f"dpd{jj}")
        nc.vector.tensor_scalar_mul(dpd[:], ysel_b[:], smalls["Dp_c"][:, jj:jj + 1])
        dpdiag.append(dpd)

    gb_out = persist.tile([128, 2], f32, tag="gb_out", name="gb_out")
    nc.vector.tensor_tensor(gb_out[:], g_msa[:], smalls["b_out_c"][:], MUL)
    gb_fc2 = persist.tile([128, 2], f32, tag="gb_fc2", name="gb_fc2")
    nc.vector.tensor_tensor(gb_fc2[:], g_mlp[:], smalls["b_fc2_c"][:], MUL)

    # ---------------- S3: xi projection into padded conv planes -----------
    # pad[j]: (128, PADW) bf16, image interior at [1+h, 1+w] (34-wide rows)
    pads = []
    for j in range(4):
        pad = work.tile([128, PADW], bf16, tag=f"pad{j}", name=f"pad{j}")
        nc.gpsimd.memset(pad[:], 0.0)
        pads.append(pad)

    for j in range(4):
        ppm = pp.tile([128, L], f32, tag="projp", name=f"pp_xiT{j}", bufs=2)
        for kk in range(2):
            for th in range(2):
                nc.tensor.matmul(
                    ppm[:, th * 512:(th + 1) * 512],
                    Wxi[kk][:, j * 128:(j + 1) * 128],
                    hT_pre[kk][:, th * 512:(th + 1) * 512],
                    start=(kk == 0), stop=(kk == 1))
        pad3 = pads[j][:, :34 * 34].rearrange("p (H W) -> p H W", H=34, W=34)
        nc.scalar.activation(pad3[:, 1:33, 1:33],
                             ppm[:, :].rearrange("p (h w) -> p h w", h=32, w=32),
                             AF.Identity, bias=smalls["b_in_xi"][:, j:j + 1])

    # z projection (row domain) -> spill to DRAM for the post phase
    for j in range(4):
        ppz = pp.tile([128, L], f32, tag="projp", name=f"pp_zTs{j}", bufs=2)
        for kk in range(2):
            for th in range(2):
                nc.tensor.matmul(
                    ppz[:, th * 512:(th + 1) * 512],
                    Wz[kk][:, j * 128:(j + 1) * 128],
                    hT_row[kk][:, th * 512:(th + 1) * 512],
                    start=(kk == 0), stop=(kk == 1))
        zt = work.tile([128, L], bf16, tag="zTs", name=f"zTs{j}", bufs=2)
        nc.scalar.activation(zt[:], ppz[:], AF.Identity, bias=smalls["b_in_z"][:, j:j + 1])
        dma(T["z_spill"][j * 128:(j + 1) * 128, :], zt[:])

    # ---------------- S4: depthwise conv 3x3 via PE diag matmuls ----------
    xsT = []
    HHALF = 16
    HFREE = HHALF * 34
    for j in range(4):
        xs = wp.tile([128, L], bf16, tag=f"xsT{j}", name=f"xsT{j}")
        for hh in range(2):
            base = hh * HHALF * 34
            cvp = pp.tile([128, HFREE], f32, tag="convp", name=f"convp{j}_{hh}", bufs=2)
            for c0 in range(0, HFREE, 512):
                c1 = min(c0 + 512, HFREE)
                for tap in range(9):
                    dy, dx = tap // 3, tap % 3
                    off = base + dy * 34 + dx
                    nc.tensor.matmul(
                        cvp[:, c0:c1], cdiag[j][tap][:],
                        pads[j][:, off + c0:off + c1],
                        start=(tap == 0), stop=(tap == 8))
            nc.scalar.activation(
                xs[:, hh * HHALF * 32:(hh + 1) * HHALF * 32]
                  .rearrange("p (h w) -> p h w", h=HHALF, w=32),
                cvp[:, :].rearrange("p (h w) -> p h w", h=HHALF, w=34)[:, :, 0:32],
                AF.Silu, bias=smalls["convb"][:, j:j + 1])
        xsT.append(xs)

    # ---------------- S5: x_dbl = [B; C] and dt_r -------------------------
    bc_t = wp.tile([128, L], bf16, tag="bc_t", name="bc_t")
    ppbc = pp.tile([128, L], f32, tag="projp", name="ppbc", bufs=2)
    for kk in range(4):
        for th in range(2):
            nc.tensor.matmul(ppbc[:, th * 512:(th + 1) * 512],
                             Wxp[kk][:, 0:128],
                             xsT[kk][:, th * 512:(th + 1) * 512],
                             start=(kk == 0), stop=(kk == 3))
    nc.scalar.copy(bc_t[:], ppbc[:])
    dtr_t = work.tile([16, L], bf16, tag="dtr_t", name="dtr_t")
    ppdtr = pp.tile([16, L], f32, tag="projp", name="ppdtr", bufs=2)
    for kk in range(4):
        for th in range(2):
            nc.tensor.matmul(ppdtr[:, th * 512:(th + 1) * 512],
                             Wxp[kk][:, 128:144],
                             xsT[kk][:, th * 512:(th + 1) * 512],
                             start=(kk == 0), stop=(kk == 3))
    nc.scalar.copy(dtr_t[:], ppdtr[:])

    # ---------------- S6: dt, -dt broadcast, w broadcast ------------------
    wbc, ndtbc = [], []
    bcast_srcs = []
    for j in range(4):
        ppd = pp.tile([128, L], f32, tag="projp", name=f"ppdt{j}", bufs=2)
        for th in range(2):
            nc.tensor.matmul(ppd[:, th * 512:(th + 1) * 512],
                             Wdt[0][:, j * 128:(j + 1) * 128],
                             dtr_t[:, th * 512:(th + 1) * 512],
                             start=True, stop=True)
        spx = work.tile([128, L], f32, tag="spx", name=f"spx{j}", bufs=2)
        nc.scalar.activation(spx[:], ppd[:], AF.Exp, bias=smalls["dtb"][:, j:j + 1])
        dt_b = work.tile([128, L], bf16, tag="dtT", name=f"dtT{j}", bufs=2)
        nc.scalar.activation(dt_b[:], spx[:], AF.Ln, bias=1.0)
        ndt = wp.tile([128, L], bf16, tag="ndtT", name=f"ndtT{j}", bufs=2)
        nc.vector.tensor_scalar_mul(ndt[:], dt_b[:], -1.0)
        w_b = wp.tile([128, L], bf16, tag="wTtmp", name=f"wT{j}", bufs=2)
        nc.vector.tensor_tensor(w_b[:], dt_b[:], xsT[j][:], MUL)
        for par in range(2):
            g = 2 * j + par
            wb = wp.tile([128, L], bf16, tag=f"wbc{g}", name=f"wbc{g}")
            wbc.append(wb)
            nb = wp.tile([128, L], bf16, tag=f"ndtbc{g}", name=f"ndtbc{g}")
            ndtbc.append(nb)
            bcast_srcs.append((wb, w_b, nb, ndt, par))

    pre_ps.close()
    workstack.close()

    # ---------------- post-phase pools (opened early: tiles live across) --
    post = ExitStack()
    pf = post.enter_context(tc.tile_pool(name="postf", bufs=1))

    ones_f = pf.tile([128, 1], f32, tag="ones_f", name="ones_f")
    nc.gpsimd.memset(ones_f[:], 1.0)
    ones_b = pf.tile([128, 1], bf16, tag="ones_b", name="ones_b")
    nc.gpsimd.memset(ones_b[:], 1.0)

    siluz = []
    for j in range(4):
        zrl = pf.tile([128, L], bf16, tag="zrl", name=f"zrl{j}", bufs=2)
        dma(zrl[:], T["z_spill"][j * 128:(j + 1) * 128, :])
        sz = pf.tile([128, L], bf16, tag="siluz", name=f"siluz{j}", bufs=4)
        nc.scalar.activation(sz[:], zrl[:], AF.Silu)
        siluz.append(sz)

    # ---------------- S7: scan (g-major) + chunked AllGather + combine ----
    scan_st = ExitStack()
    yps = scan_st.enter_context(tc.tile_pool(name="yps", bufs=1, space="PSUM"))
    spool = scan_st.enter_context(tc.tile_pool(name="spool", bufs=2))
    bpool = scan_st.enter_context(tc.tile_pool(name="bpool", bufs=1))

    ssum = yps.tile([128, 8], f32, tag="ssum", name="ssum", bufs=1)
    ssq = yps.tile([128, 8], f32, tag="ssq", name="ssq", bufs=1)
    y_row = [None] * 4
    sq_y = [None] * 4
    yrps = [None] * 4

    # staggered post-weight loads: (dram, r0, r, h0, h1, dest_tile) pairs
    wload_specs = []
    for nm, dram, rows, cols in (("Wout", T["W_out"], DI, DIM),
                                 ("Wfc1", T["W_fc1"], DIM, HID),
                                 ("Wfc2", T["W_fc2"], HID, DIM)):
        n = (rows + 127) // 128
        tiles = []
        for jj in range(n):
            tb = pf.tile([128, cols], bf16, tag=f"{nm}_b{jj}", name=f"{nm}_b{jj}")
            tiles.append(tb)
            for h0 in range(0, cols, 512):
                wload_specs.append((dram, jj * 128, h0, min(h0 + 512, cols), tb))
        if nm == "Wout":
            Wout = tiles
        elif nm == "Wfc1":
            Wfc1 = tiles
        else:
            Wfc2 = tiles
    wload_pending = []
    wload_i = [0]

    def emit_wloads(ndma):
        for tf, tb, h0, h1 in wload_pending:
            nc.scalar.copy(tb[:, h0:h1], tf[:, :h1 - h0])
        wload_pending.clear()
        for _ in range(ndma):
            if wload_i[0] >= len(wload_specs):
                return
            dram, r0, h0, h1, tb = wload_specs[wload_i[0]]
            wload_i[0] += 1
            tf = pf.tile([128, 512], f32, tag="pstage", name=f"pst_w{wload_i[0]}", bufs=2)
            dma(tf[:, :h1 - h0], dram[r0:r0 + 128, h0:h1])
            wload_pending.append((tf, tb, h0, h1))

    def emit_combine(g, anchor=None, mk=None):
        j, par = g // 2, g % 2
        if par == 0:
            y_row[j] = pf.tile([128, L], bf16, tag="yrow", name=f"yrow{j}", bufs=4)
            sq_y[j] = pf.tile([128, L], bf16, tag="sqy", name=f"sq_y{j}", bufs=2)
        yr, sq = y_row[j], sq_y[j]
        r0, r1 = par * 64, par * 64 + 64
        ysk = []
        for k in range(4):
            t16 = pf.tile([64, L], bf16, tag="ysk", name=f"ysk{g}_{k}", bufs=4)
            ld = dma(t16[:], T[f"ys_g{g}"][k, :, :])
            if anchor is not None:
                _add_dep_helper(ld.ins, anchor.ins, sync=True,
                                reason="defer gather read past next group")
            ysk.append(t16)
        yrp = yrps[j]
        views = [
            ysk[0][:, :],
            (ysk[1][:, :].rearrange("p (w h) -> p w h", w=32, h=32)
                         .rearrange("p w h -> p h w")),
            ysk[2][:, ::-1],
            (ysk[3][:, ::-1].rearrange("p (w h) -> p w h", w=32, h=32)
                            .rearrange("p w h -> p h w")),
        ]
        for th in range(2):
            for k in range(4):
                v = views[k]
                if len(v.shape) == 3:
                    vth = v[:, th * 16:(th + 1) * 16, :]
                else:
                    vth = v[:, th * 512:(th + 1) * 512]
                nc.tensor.matmul(
                    yrp[r0:r1, th * 512:(th + 1) * 512],
                    identb[0:64, 0:64], vth,
                    start=(k == 0), stop=(k == 3))
        if mk is not None:
            nc.scalar.activation(yr[r0:r1, :], yrp[r0:r1, :], AF.Identity,
                                 bias=mk[r0:r1, 0:1])
            nc.scalar.activation(sq[r0:r1, :], yrp[r0:r1, :], AF.Square,
                                 bias=mk[r0:r1, 0:1])
        else:
            nc.scalar.copy(yr[r0:r1, :], yrp[r0:r1, :])
            nc.scalar.activation(sq[r0:r1, :], yrp[r0:r1, :], AF.Square)
        for tb in range(8):
            nc.tensor.matmul(ssum[:, tb:tb + 1],
                             yr[r0:r1, tb * 128:(tb + 1) * 128],
                             ones_b[r0:r1, :],
                             start=(g == 0), stop=(g == 7))
            nc.tensor.matmul(ssq[:, tb:tb + 1],
                             sq[r0:r1, tb * 128:(tb + 1) * 128],
                             ones_b[r0:r1, :],
                             start=(g == 0), stop=(g == 7))

    it = 0
    pending_g = []
    ys16_insts = [None] * 8
    mk_tiles = [None] * 8
    from concourse.bass import _add_dep_helper

    def emit_bcast(g):
        wb, w_b, nb, ndt, par = bcast_srcs[g]
        wsrc = w_b[par * 64:par * 64 + 64, :]
        dma(wb[0:64, :], wsrc)
        dma(wb[64:128, :], wsrc)
        nsrc = ndt[par * 64:par * 64 + 64, :]
        dma(nb[0:64, :], nsrc)
        dma(nb[64:128, :], nsrc)
    for g in range(2):
        emit_bcast(g)

    def scan_it(j, par, ch, il, ypt, Bb, Cb):
        nonlocal_it = scan_it.it
        g = 2 * j + par
        i = ch * CHUNK + il
        dA = spool.tile([128, L], bf16, tag="dA", name=f"dA{nonlocal_it}", bufs=3)
        nc.scalar.activation(dA[:], ndtbc[g][:], AF.Exp,
                             scale=acols[:, g * 32 + i:g * 32 + i + 1])
        if ch == 1 and il == 0:
            mkt = spool.tile([128, 1], bf16, tag="mk", name=f"mk{g}", bufs=8)
            nc.scalar.activation(mkt[:, 0:1], dA[:, 0:1], AF.Identity, scale=0.0)
            mk_tiles[g] = mkt
        xin = spool.tile([128, L], bf16, tag="xin", name=f"xin{nonlocal_it}", bufs=3)
        xeng = nc.vector if (nonlocal_it % 7 == 0) else nc.gpsimd
        xeng.tensor_tensor(xin[:], wbc[g][:], Bb[il][:], MUL)
        h = spool.tile([128, L], bf16, tag="h", name=f"h{nonlocal_it}", bufs=3)
        nc.vector.tensor_tensor_scan(h[:], dA[:], xin[:], 0.0, MUL, ADD)
        yc = spool.tile([128, L], bf16, tag="yc", name=f"yc{nonlocal_it}", bufs=3)
        nc.vector.tensor_tensor(yc[:], h[:], Cb[il][:], MUL)
        for th in range(2):
            nc.tensor.matmul(
                ypt[par * 64:par * 64 + 64, th * 512:(th + 1) * 512],
                ysel_b[:], yc[:, th * 512:(th + 1) * 512],
                start=(ch == 0 and il == 0), stop=False)
        scan_it.it += 1
    scan_it.it = 0

    def load_bc(j, ch, sfx=""):
        Bb, Cb = [], []
        for il in range(CHUNK):
            i = ch * CHUNK + il
            bbt = bpool.tile([128, L], bf16, tag="Bb", name=f"Bb{sfx}{j}_{ch}_{il}",
                             bufs=2 * CHUNK - 2)
            dma(bbt[:], bc_t[2 * i:2 * i + 2, :]
                .partition_broadcast(64).rearrange("d n f -> n d f"))
            Bb.append(bbt)
            cbt = bpool.tile([128, L], bf16, tag="Cb", name=f"Cb{sfx}{j}_{ch}_{il}",
                             bufs=2 * CHUNK - 2)
            dma(cbt[:], bc_t[64 + 2 * i:64 + 2 * i + 2, :]
                .partition_broadcast(64).rearrange("d n f -> n d f"))
            Cb.append(cbt)
        return Bb, Cb

    def emit_tail(j, par, ypt):
        g = 2 * j + par
        r0, r1 = par * 64, par * 64 + 64
        for th in range(2):
            nc.tensor.matmul(
                ypt[r0:r1, th * 512:(th + 1) * 512],
                dpdiag[j][r0:r1, :], xsT[j][r0:r1, th * 512:(th + 1) * 512],
                start=False, stop=(th == 1))
        ysb16 = spool.tile([64, L], bf16, tag="ys16", name=f"ys16_{g}", bufs=1)
        ys16_insts[g] = nc.scalar.copy(ysb16[:], ypt[r0:r1, :])
        dma(T["ys_lb"][g * 64:(g + 1) * 64, :], ysb16[:])
        nc.gpsimd.collective_compute(
            "AllGather", BYP,
            replica_groups=[[0, 1, 2, 3], [4, 5, 6, 7]],
            ins=[T["ys_lb"][g * 64:(g + 1) * 64, :]],
            outs=[T[f"ys_g{g}"][:, :, :]],
        )
        pending_g.append(g)

    for j in range(3):
        ypt = yps.tile([128, L], f32, tag="ypt", name=f"ypt{j}", bufs=2)
        yrps[j] = yps.tile([128, L], f32, tag="yrp", name=f"yrp{j}", bufs=1)
        if j > 0:
            emit_bcast(2 * j)
            emit_bcast(2 * j + 1)
        for par in range(2):
            for ch in range(NCHUNK):
                if ch in (2, 5) and len(pending_g) > 1:
                    gq = pending_g.pop(0)
                    emit_combine(gq, anchor=ys16_insts[gq + 1],
                                 mk=mk_tiles[gq + 2] if gq + 2 < 8 else None)
                if ch in (3, 6):
                    emit_wloads(1)
                Bb, Cb = load_bc(j, ch, sfx=f"q{par}_")
                for il in range(CHUNK):
                    scan_it(j, par, ch, il, ypt, Bb, Cb)
            emit_tail(j, par, ypt)

    # last j: g-major so the two final gathers stagger
    emit_bcast(6)
    emit_bcast(7)
    j = 3
    ypt = yps.tile([128, L], f32, tag="ypt", name=f"ypt{j}", bufs=2)
    yrps[j] = yps.tile([128, L], f32, tag="yrp", name=f"yrp{j}", bufs=1)
    for par in range(2):
        for ch in range(NCHUNK):
            if ch in (2, 5) and len(pending_g) > 1:
                gq = pending_g.pop(0)
                emit_combine(gq, anchor=ys16_insts[gq + 1],
                             mk=mk_tiles[gq + 2] if gq + 2 < 8 else None)
            if ch in (3, 6):
                emit_wloads(1)
            Bb, Cb = load_bc(j, ch, sfx=f"p{par}_")
            for il in range(CHUNK):
                scan_it(j, par, ch, il, ypt, Bb, Cb)
        emit_tail(j, par, ypt)

    while pending_g:
        emit_wloads(0)
        gq = pending_g.pop(0)
        emit_combine(gq, anchor=ys16_insts[gq + 1] if gq + 1 < 8 else None,
                     mk=mk_tiles[gq + 2] if gq + 2 < 8 else None)
    emit_wloads(16)
    emit_wloads(0)

    # ---------------- S12: LN over d + ln_w/ln_b, gate --------------------
    def stat_cols(ssum_t, ssq_t, dim, name):
        mu = pf.tile([128, 8], f32, tag="pmu", name=f"pmu_{name}", bufs=2)
        nc.vector.tensor_scalar_mul(mu[:], ssum_t[:], 1.0 / dim)
        mu2 = pf.tile([128, 8], f32, tag="pmu2", name=f"pmu2_{name}", bufs=2)
        nc.vector.tensor_tensor(mu2[:], mu[:], mu[:], MUL)
        var = pf.tile([128, 8], f32, tag="pvar", name=f"pvar_{name}", bufs=2)
        nc.vector.scalar_tensor_tensor(var[:], ssq_t[:], 1.0 / dim, mu2[:], MUL, SUB)
        std = pf.tile([128, 8], f32, tag="pstd", name=f"pstd_{name}", bufs=2)
        nc.scalar.activation(std[:], var[:], AF.Sqrt, bias=eps_col[:, 0:1])
        rstd = pf.tile([128, 8], f32, tag="prstd", name=f"prstd_{name}", bufs=2)
        nc.vector.reciprocal(rstd[:], std[:])
        return mu, rstd

    mu_y, rstd_y = stat_cols(ssum, ssq, DI, "y")

    scan_st.close()

    ppost = post.enter_context(tc.tile_pool(name="ps_post", bufs=2, space="PSUM"))
    ymu_bc = bcast_cols(mu_y, "Pym", pf, "bcA", ppost, "php", dt_out=bf16)
    yrstd_bc = bcast_cols(rstd_y, "Pyr", pf, "bcB", ppost, "php", dt_out=bf16)

    gated = []
    for j in range(4):
        t1 = pf.tile([128, L], bf16, tag="psc", name=f"lny1_{j}", bufs=2)
        nc.vector.tensor_tensor(t1[:], y_row[j][:], ymu_bc[:], SUB)
        t2 = pf.tile([128, L], bf16, tag="psc", name=f"lny2_{j}", bufs=2)
        nc.vector.tensor_tensor(t2[:], t1[:], yrstd_bc[:], MUL)
        t3 = pf.tile([128, L], bf16, tag="psc", name=f"lny3_{j}", bufs=2)
        nc.scalar.activation(t3[:], t2[:], AF.Identity,
                             bias=smalls["lnb_c"][:, j:j + 1],
                             scale=smalls["lnw_c"][:, j:j + 1])
        gt = pf.tile([128, L], bf16, tag=f"gated{j}", name=f"gated{j}")
        geng = nc.gpsimd if (j % 2 == 1) else nc.vector
        geng.tensor_tensor(gt[:], t3[:], siluz[j][:], MUL)
        gated.append(gt)

    # ---------------- S14: hy, residual -----------------------------------
    xTrs = []
    for cc in range(2):
        xt = pf.tile([128, L], f32, tag="xTrl", name=f"xTr{cc}", bufs=2)
        dma(xt[:], T["xT_row"][cc * 128:(cc + 1) * 128, :])
        xTrs.append(xt)
    x2T = []
    for cc in range(2):
        php = ppost.tile([128, L], f32, tag="php", name=f"php{cc}", bufs=2)
        for kk in range(4):
            for th in range(2):
                nc.tensor.matmul(php[:, th * 512:(th + 1) * 512],
                                 Wout[kk][:, cc * 128:(cc + 1) * 128],
                                 gated[kk][:, th * 512:(th + 1) * 512],
                                 start=(kk == 0), stop=(kk == 3))
        t1 = pf.tile([128, L], f32, tag="pscf", name=f"hyg{cc}", bufs=2)
        nc.scalar.activation(t1[:], php[:], AF.Identity,
                             bias=gb_out[:, cc:cc + 1], scale=g_msa[:, cc:cc + 1])
        x2 = pf.tile([128, L], bf16, tag=f"x2T{cc}", name=f"x2T{cc}")
        with nc.allow_low_precision(reason="x2 residual in bf16; tol 2e-2"):
            nc.vector.tensor_tensor(x2[:], t1[:], xTrs[cc][:], ADD)
        x2T.append(x2)

    # ---------------- S15: LN2 + modulate ---------------------------------
    x2sum = ppost.tile([128, 8], f32, tag="pfc2p", name="x2sum", bufs=2)
    x2sq = ppost.tile([128, 8], f32, tag="pfc2p", name="x2sq", bufs=2)
    sqx = []
    for ccj in range(2):
        sq = pf.tile([128, L], bf16, tag="sqt", name=f"sq_x2{ccj}", bufs=2)
        nc.scalar.activation(sq[:], x2T[ccj][:], AF.Square)
        sqx.append(sq)
    for tb in range(8):
        for ccj in range(2):
            nc.tensor.matmul(x2sum[:, tb:tb + 1],
                             x2T[ccj][:, tb * 128:(tb + 1) * 128], ones_b[:],
                             start=(ccj == 0), stop=(ccj == 1))
            nc.tensor.matmul(x2sq[:, tb:tb + 1],
                             sqx[ccj][:, tb * 128:(tb + 1) * 128], ones_b[:],
                             start=(ccj == 0), stop=(ccj == 1))
    mu_x2, rstd_x2 = stat_cols(x2sum, x2sq, DIM, "x2")
    x2mu_bc = bcast_cols(mu_x2, "Px2m", pf, "bcA", ppost, "php", dt_out=bf16)
    x2rstd_bc = bcast_cols(rstd_x2, "Px2r", pf, "bcB", ppost, "php", dt_out=bf16)
    mT = []
    for cc in range(2):
        t1 = pf.tile([128, L], bf16, tag="psc", name=f"m1_{cc}", bufs=2)
        nc.vector.tensor_tensor(t1[:], x2T[cc][:], x2mu_bc[:], SUB)
        t2 = pf.tile([128, L], bf16, tag="psc", name=f"m2_{cc}", bufs=2)
        nc.vector.tensor_tensor(t2[:], t1[:], x2rstd_bc[:], MUL)
        mb = pf.tile([128, L], bf16, tag=f"mT{cc}", name=f"mT{cc}")
        nc.scalar.activation(mb[:], t2[:], AF.Identity,
                             bias=sh_mlp[:, cc:cc + 1], scale=s1_mlp[:, cc:cc + 1])
        mT.append(mb)

    # ---------------- S16: MLP + final residual ---------------------------
    pfc2ps = [ppost.tile([128, L], f32, tag="pfc2p", name=f"pfc2_{cc}", bufs=2)
              for cc in range(2)]
    for kk in range(8):
        pfc = ppost.tile([128, L], f32, tag="php", name=f"pfc1_{kk}", bufs=2)
        for kk2 in range(2):
            for th in range(2):
                nc.tensor.matmul(pfc[:, th * 512:(th + 1) * 512],
                                 Wfc1[kk2][:, kk * 128:(kk + 1) * 128],
                                 mT[kk2][:, th * 512:(th + 1) * 512],
                                 start=(kk2 == 0), stop=(kk2 == 1))
        gl = pf.tile([128, L], bf16, tag="gelu", name=f"gelu{kk}", bufs=2)
        nc.scalar.activation(gl[:], pfc[:], AF.Gelu_apprx_tanh, bias=smalls["b_fc1_c"][:, kk:kk + 1])
        for cc in range(2):
            for th in range(2):
                nc.tensor.matmul(pfc2ps[cc][:, th * 512:(th + 1) * 512],
                                 Wfc2[kk][:, cc * 128:(cc + 1) * 128],
                                 gl[:, th * 512:(th + 1) * 512],
                                 start=(kk == 0), stop=(kk == 7))

    for cc in range(2):
        t1 = pf.tile([128, L], f32, tag="pscf", name=f"mlpg{cc}", bufs=2)
        nc.scalar.activation(t1[:], pfc2ps[cc][:], AF.Identity,
                             bias=gb_fc2[:, cc:cc + 1], scale=g_mlp[:, cc:cc + 1])
        o = pf.tile([128, L], f32, tag="outTt", name=f"outT{cc}", bufs=1)
        nc.vector.tensor_tensor(o[:], t1[:], x2T[cc][:], ADD)
        dma(T["outT"][cc * 128:(cc + 1) * 128, :], o[:])

    post.close()
    wstack.close()
    perstack.close()


# ---------------------------------------------------------------------------
# Host side
_PROGRAM = None


def _get_program():
    global _PROGRAM
    if _PROGRAM is None:
        _PROGRAM = build_program()
    return _PROGRAM


def _q_img(x, k):
    img = x.reshape(Hs, Ws, -1)
    if k == 0:
        out = img
    elif k == 1:
        out = img.transpose(1, 0, 2)
    elif k == 2:
        out = img[::-1, ::-1]
    else:
        out = img.transpose(1, 0, 2)[::-1, ::-1]
    return np.ascontiguousarray(out.reshape(L, -1))


def _conv_w_q(w, k):
    if k == 0:
        return w
    if k == 1:
        return np.ascontiguousarray(w.transpose(1, 0, 2))
    if k == 2:
        return np.ascontiguousarray(w[::-1, ::-1])
    return np.ascontiguousarray(w.transpose(1, 0, 2)[::-1, ::-1])


def _col128(v, ncols):
    return np.ascontiguousarray(v.reshape(ncols, 128).T)


def prep_inputs(inputs):
    inp = {k: np.asarray(v, dtype=np.float32) for k, v in inputs.items()}
    x, c = inp["x"], inp["c"]

    shared = {}
    shared["W_ada"] = inp["W_ada"]
    shared["b_ada"] = inp["b_ada"].reshape(1, 6 * DIM)
    W_in = inp["W_in"]
    shared["W_in_xi"] = np.ascontiguousarray(W_in[:, :DI])
    shared["W_in_z"] = np.ascontiguousarray(W_in[:, DI:])
    b_in = inp["b_in"]
    shared["b_in_xi"] = _col128(b_in[:DI], 4)
    shared["b_in_z"] = _col128(b_in[DI:], 4)
    shared["lnw_c"] = _col128(inp["ln_w"], 4)
    shared["lnb_c"] = _col128(inp["ln_b"], 4)
    shared["W_out"] = inp["W_out"]
    shared["b_out_c"] = _col128(inp["b_out"], 2)
    shared["W_fc1"] = inp["W_fc1"]
    shared["b_fc1_c"] = _col128(inp["b_fc1"], 8)
    shared["W_fc2"] = inp["W_fc2"]
    shared["b_fc2_c"] = _col128(inp["b_fc2"], 2)
    shared["ident"] = np.eye(128, dtype=np.float32)
    p = np.arange(128)
    ys = np.zeros((128, 64), np.float32)
    ys[p, p % 64] = 1.0
    shared["ysel"] = ys

    in_maps = []
    for core in range(8):
        b, k = core // 4, core % 4
        m = dict(shared)
        xb = x[b]
        xpre = _q_img(xb, k)
        m["x_pre"] = xpre
        m["xT_pre"] = np.ascontiguousarray(xpre.T)
        m["x_row"] = np.ascontiguousarray(xb)
        m["xT_row"] = np.ascontiguousarray(xb.T)
        m["c_vec"] = c[b].reshape(1, DIM)

        cw = _conv_w_q(inp["conv_w"].reshape(3, 3, DI), k)
        cwt = cw.reshape(9, 4, 128)                       # [tap, dtile, p]
        m["convw"] = np.ascontiguousarray(cwt.transpose(2, 1, 0).reshape(128, 36))
        m["convb"] = _col128(inp["conv_b"], 4)

        Wxp = inp["W_xproj"][k]                           # (DI, 144) cols [dtr,B,C]
        m["W_xp"] = np.ascontiguousarray(
            np.concatenate([Wxp[:, DTR:DTR + DS], Wxp[:, DTR + DS:], Wxp[:, :DTR]],
                           axis=1))
        m["W_dtm"] = np.ascontiguousarray(inp["W_dt"][k])
        m["dtb"] = _col128(inp["dt_bias"][k], 4)
        m["Dp_c"] = _col128(inp["Dp"][k], 4)

        alog = inp["A_log"][k]                            # (DI, DS)
        acols = np.zeros((128, 256), np.float32)
        for g in range(8):
            for i in range(NPAIRS):
                acols[:, g * 32 + i] = alog[g * 64 + (p % 64), 2 * i + (p // 64)]
        m["a_logcols"] = acols
        in_maps.append(m)
    return in_maps


def kernel(**inputs):
    nc = _get_program()
    in_maps = prep_inputs(inputs)
    res = run_bass_kernel_spmd(nc, in_maps, list(range(8)))
    out = np.zeros((B, L, DIM), np.float32)
    for b in range(B):
        out[b] = res.results[4 * b]["outT"].T
    return out


# revision 92
# speedup vs baseline: 1.2227x; 1.0073x over previous
"""DiM block (Mamba-style selective-scan transformer block) on 8 TRN2 cores.

Sharding: core i handles (b = i//4, k = i%4) — one batch sample and one of
the 4 scan directions. The spatial permutation q_k (identity / transpose /
flip / anti-transpose) is pushed onto host-prepared inputs (x pre-permuted,
conv taps transformed) so ONE SPMD program serves all 8 cores. The per-sample
combine over k happens on-device via per-d-group AllGathers within each
4-core group (chunked so they overlap the scan), followed by compile-time
(uniform) re-orderings done as identity-matmuls with permuted moving APs.

Scan layout: partitions = (n-pair:2 outer, d:64 inner), free = t.
Per (d-group g, n-pair i): ACT computes dA = exp(A*dt) from a pre-broadcast
-dt tile, DVE does the w*B and h*C multiplies (bf16 2x mode), the hardware
scan runs mostly on GPSIMD (1/8 on DVE to balance), PE reduces the n-pair
into a per-d-group PSUM y accumulator that lives across the whole group.
"""
import json
import sys

sys.path.insert(0, "/opt/trn_rl_repo")

import numpy as np
import concourse.bass as bass
import concourse.mybir as mybir
import concourse.tile as tile
from concourse.bass_utils import run_bass_kernel_spmd

# ---------------------------------------------------------------------------
# Workaround: this walrus build rejects instructions carrying >1 embedded
# sem-wait. Split extra waits onto same-engine NoOps at BIR serialization.
_MAXW = 1
_wsplit_counter = [0]


def _split_multi_waits(bir: dict) -> dict:
    for fn in bir.get("functions", []):
        for bb in fn.get("blocks", []):
            insts = bb.get("instructions", [])
            if not any(
                len((i.get("sync_info") or {}).get("on_wait") or []) > _MAXW
                for i in insts
            ):
                continue
            out = []
            for inst in insts:
                si = inst.get("sync_info")
                waits = (si or {}).get("on_wait") or []
                if len(waits) > _MAXW and inst.get("engine"):
                    for w in waits[:-_MAXW]:
                        _wsplit_counter[0] += 1
                        out.append({
                            "debug": inst.get("debug", 0),
                            "engine": inst["engine"],
                            "ins": [], "outs": [],
                            "name": f"I-wsplit-{_wsplit_counter[0]}",
                            "opcode": "NoOp",
                            "sync_info": {"on_update": [], "on_wait": [w]},
                        })
                    si["on_wait"] = waits[-_MAXW:]
                out.append(inst)
            bb["instructions"] = out
    return bir


_orig_to_json_bytes = bass.Bass.to_json_bytes


def _patched_to_json_bytes(self) -> bytes:
    j = json.loads(_orig_to_json_bytes(self))
    _split_multi_waits(j)
    return json.dumps(j).encode()


bass.Bass.to_json_bytes = _patched_to_json_bytes

# ---------------------------------------------------------------------------
B, Hs, Ws, DIM = 2, 32, 32, 256
L = Hs * Ws
DI = 2 * DIM
DS = 64
DTR = DIM // 16
K = 4
HID = 4 * DIM

f32 = mybir.dt.float32
bf16 = mybir.dt.bfloat16
MUL = mybir.AluOpType.mult
ADD = mybir.AluOpType.add
SUB = mybir.AluOpType.subtract
BYP = mybir.AluOpType.bypass
AF = mybir.ActivationFunctionType
AX = mybir.AxisListType

EPS = 1e-6
NPAIRS = DS // 2          # 32 n-pairs
CHUNK = 4                 # n-pairs per chunk
NCHUNK = NPAIRS // CHUNK  # 8
PADW = 34 * 34 + 8        # padded conv plane + slack for shifted windows
CFREE = 1088              # conv output plane (32 rows x 34 cols)


def build_program(dve_scan_mod=0, pe_combine=True):
    nc = bass.Bass()

    def din(name, shape, dt=f32):
        return nc.dram_tensor(name, list(shape), dt, kind="ExternalInput")

    T = {}
    T["x_pre"] = din("x_pre", (L, DIM))
    T["xT_pre"] = din("xT_pre", (DIM, L))
    T["x_row"] = din("x_row", (L, DIM))
    T["xT_row"] = din("xT_row", (DIM, L))
    T["c_vec"] = din("c_vec", (1, DIM))
    T["W_ada"] = din("W_ada", (DIM, 6 * DIM))
    T["b_ada"] = din("b_ada", (1, 6 * DIM))
    T["W_in_xi"] = din("W_in_xi", (DIM, DI))
    T["W_in_z"] = din("W_in_z", (DIM, DI))
    T["b_in_xi"] = din("b_in_xi", (128, 4))
    T["b_in_z"] = din("b_in_z", (128, 4))
    T["convw"] = din("convw", (128, 36))      # [d_part, dtile*9 + tap]
    T["convb"] = din("convb", (128, 4))
    T["W_xp"] = din("W_xp", (DI, 144))        # cols reordered [B(64), C(64), dtr(16)]
    T["W_dtm"] = din("W_dtm", (DTR, DI))
    T["dtb"] = din("dtb", (128, 4))
    T["a_logcols"] = din("a_logcols", (128, 256))  # [p, g*32+i] = A_log[k, g*64+p%64, 2i+p//64]
    T["ysel"] = din("ysel", (128, 64))        # [p, d] = (p%64 == d)
    T["Dp_c"] = din("Dp_c", (128, 4))
    T["lnw_c"] = din("lnw_c", (128, 4))
    T["lnb_c"] = din("lnb_c", (128, 4))
    T["W_out"] = din("W_out", (DI, DIM))
    T["b_out_c"] = din("b_out_c", (128, 2))
    T["W_fc1"] = din("W_fc1", (DIM, HID))
    T["b_fc1_c"] = din("b_fc1_c", (128, 8))
    T["W_fc2"] = din("W_fc2", (HID, DIM))
    T["b_fc2_c"] = din("b_fc2_c", (128, 2))
    T["ident"] = din("ident", (128, 128))

    T["outT"] = nc.dram_tensor("outT", [DIM, L], f32, kind="ExternalOutput")
    T["ys_lb"] = nc.dram_tensor("ys_lb", [DI, L], bf16)
    for g in range(8):
        T[f"ys_g{g}"] = nc.dram_tensor(f"ys_g{g}", [4, 64, L], bf16)
    T["z_spill"] = nc.dram_tensor("z_spill", [DI, L], bf16)

    with tile.TileContext(nc) as tc:
        _build_body(nc, tc, T, dve_scan_mod, pe_combine)
    return nc


def _build_body(nc, tc, T, dve_scan_mod, pe_combine):
    from contextlib import ExitStack

    dma = nc.sync.dma_start

    perstack = ExitStack()
    persist = perstack.enter_context(tc.tile_pool(name="persist", bufs=1))
    wstack = ExitStack()
    wp = wstack.enter_context(tc.tile_pool(name="weights", bufs=1))
    workstack = ExitStack()
    work = workstack.enter_context(tc.tile_pool(name="work", bufs=1))
    pre_ps = ExitStack()
    pp = pre_ps.enter_context(tc.tile_pool(name="ps_pre", bufs=2, space="PSUM"))

    # ---------------- early x loads (these gate the pre-phase chain) ------
    xstat_tiles = {}
    tl = []
    for j in range(8):
        xt = work.tile([128, DIM], f32, tag="lnx", name=f"lnx_pre{j}", bufs=8)
        dma(xt[:], T["x_pre"][j * 128:(j + 1) * 128, :])
        tl.append(xt)
    xstat_tiles["x_pre"] = tl

    # ---------------- S0: load + cast weights (staged through 2 slots) ----
    def load_cast(dram, rows, cols, name, dest=None):
        dest = dest or work
        n = (rows + 127) // 128
        outs = []
        for j in range(n):
            r0 = j * 128
            r = min(128, rows - r0)
            tf = work.tile([r, cols], f32, tag="stage", name=f"st_{name}{j}", bufs=2)
            dma(tf[:], dram[r0:r0 + r, :])
            tb = dest.tile([r, cols], bf16, tag=f"{name}_b{j}", name=f"{name}_b{j}")
            nc.gpsimd.tensor_copy(tb[:], tf[:])
            outs.append(tb)
        return outs

    identf = persist.tile([128, 128], f32, tag="identf", name="identf")
    dma(identf[:], T["ident"][:, :])
    identb = persist.tile([128, 128], bf16, tag="identb", name="identb")
    nc.gpsimd.tensor_copy(identb[:], identf[:])

    Wada = []
    for j in range(2):
        tb = work.tile([128, 6 * DIM], bf16, tag=f"Wada_b{j}", name=f"Wada_b{j}")
        for hcol in range(2):
            tf = work.tile([128, 1024], f32, tag="stage", name=f"st_Wada{j}_{hcol}", bufs=2)
            dma(tf[:, :768], T["W_ada"][j * 128:(j + 1) * 128, hcol * 768:(hcol + 1) * 768])
            nc.vector.tensor_copy(tb[:, hcol * 768:(hcol + 1) * 768], tf[:, :768])
        Wada.append(tb)
    eps_col = persist.tile([128, 1], f32, tag="eps_col", name="eps_col")
    nc.gpsimd.memset(eps_col[:], EPS)
    # ---------------- S1: adaLN modulation vector -------------------------
    c_t = work.tile([1, DIM], f32, tag="c_t", name="c_t")
    dma(c_t[:], T["c_vec"][:, :])
    c_silu = work.tile([1, DIM], f32, tag="c_silu", name="c_silu")
    nc.scalar.activation(c_silu[:], c_t[:], AF.Silu)
    c_colp = pp.tile([128, 2], f32, tag="projp", name="c_colp", bufs=2)
    for half in range(2):
        nc.tensor.transpose(c_colp[:, half:half + 1],
                            c_silu[0:1, half * 128:(half + 1) * 128],
                            identf[0:1, 0:1])
    c_colb = work.tile([128, 2], bf16, tag="c_colb", name="c_colb")
    nc.scalar.copy(c_colb[:], c_colp[:])

    mod = work.tile([1, 6 * DIM], f32, tag="mod", name="mod")
    for fb in range(3):
        bada_s = work.tile([1, 512], f32, tag="bada_s", name=f"bada_s{fb}", bufs=2)
        dma(bada_s[:], T["b_ada"][0:1, fb * 512:(fb + 1) * 512])
        pmod = pp.tile([1, 512], f32, tag="projp", name=f"pmod{fb}", bufs=2)
        for kk in range(2):
            nc.tensor.matmul(pmod[:], c_colb[:, kk:kk + 1],
                             Wada[kk][:, fb * 512:(fb + 1) * 512],
                             start=(kk == 0), stop=(kk == 1))
        nc.vector.tensor_tensor(mod[:, fb * 512:(fb + 1) * 512], pmod[:],
                                bada_s[:], ADD)

    mcols = []
    for i6 in range(6):
        mcp = pp.tile([128, 2], f32, tag="projp", name=f"mcolp{i6}", bufs=2)
        for half in range(2):
            nc.tensor.transpose(
                mcp[:, half:half + 1],
                mod[0:1, i6 * 256 + half * 128:i6 * 256 + (half + 1) * 128],
                identf[0:1, 0:1])
        col = persist.tile([128, 2], f32, tag=f"mcol{i6}", name=f"mcol{i6}")
        nc.scalar.copy(col[:], mcp[:])
        mcols.append(col)
    sh_msa, sc_msa, g_msa, sh_mlp, sc_mlp, g_mlp = mcols
    s1_msa = persist.tile([128, 2], f32, tag="s1_msa", name="s1_msa")
    nc.scalar.activation(s1_msa[:], sc_msa[:], AF.Identity, bias=1.0)
    s1_mlp = persist.tile([128, 2], f32, tag="s1_mlp", name="s1_mlp")
    nc.scalar.activation(s1_mlp[:], sc_mlp[:], AF.Identity, bias=1.0)

    # ---------------- shared helpers -------------------------------------
    def ln_stats(xtiles, name, pool, x_dram=None):
        mu = pool.tile([128, 8], f32, tag=f"mu_{name}", name=f"mu_{name}")
        sq = pool.tile([128, 8], f32, tag=f"sq_{name}", name=f"sq_{name}")
        for j in range(8):
            if xtiles is None:
                xt = pool.tile([128, DIM], f32, tag="lnxr", name=f"lnxr_{name}{j}", bufs=2)
                dma(xt[:], x_dram[j * 128:(j + 1) * 128, :])
            else:
                xt = xtiles[j]
            nc.vector.tensor_reduce(mu[:, j:j + 1], xt[:], AX.X, ADD)
            scr = pool.tile([128, DIM], f32, tag="lnscr", name=f"lnscr_{name}{j}", bufs=2)
            nc.scalar.activation(scr[:], xt[:], AF.Square, accum_out=sq[:, j:j + 1])
        nc.vector.tensor_scalar_mul(mu[:], mu[:], 1.0 / DIM)
        mu2 = pool.tile([128, 8], f32, tag="ln_mu2", name=f"mu2_{name}")
        nc.vector.tensor_tensor(mu2[:], mu[:], mu[:], MUL)
        var = pool.tile([128, 8], f32, tag="ln_var", name=f"var_{name}")
        nc.vector.scalar_tensor_tensor(var[:], sq[:], 1.0 / DIM, mu2[:], MUL, SUB)
        std = pool.tile([128, 8], f32, tag="ln_std", name=f"std_{name}")
        nc.scalar.activation(std[:], var[:], AF.Sqrt, bias=eps_col[:, 0:1])
        rstd = pool.tile([128, 8], f32, tag=f"rstd_{name}", name=f"rstd_{name}")
        nc.vector.reciprocal(rstd[:], std[:])
        return mu, rstd

    def bcast_cols(stat, name, pool, tag, psum_pool, ptag, dt_out=f32):
        """(128,8) per-token stat -> (128,1024) all-partition broadcast tile."""
        statT_p = psum_pool.tile([8, 128], f32, tag=ptag, name=f"sTp_{name}", bufs=2)
        nc.tensor.transpose(statT_p[:], stat[:], identf[:])
        statT = pool.tile([8, 128], dt_out, tag="statT", name=f"sT_{name}", bufs=2)
        nc.scalar.copy(statT[:], statT_p[:])
        row2 = pool.tile([2, L], dt_out, tag=f"row2{dt_out}", name=f"r2_{name}", bufs=2)
        dma(row2[0:1, :], statT[:, :])
        dma(row2[1:2, :], statT[:, :])
        bc = pool.tile([128, L], dt_out, tag=tag, name=f"bc_{name}", bufs=1)
        dma(bc[:], row2[:, :].partition_broadcast(64).rearrange("n d f -> d n f"))
        return bc

    def make_hT(xT_dram, mu, rstd, name, pool):
        mu_bc = bcast_cols(mu, f"mu{name}", pool, "bcA", pp, "projp", dt_out=f32)
        rstd_bc = bcast_cols(rstd, f"rs{name}", pool, "bcB", pp, "projp", dt_out=bf16)
        outs = []
        for cc in range(2):
            xt = pool.tile([128, L], f32, tag="hscr", name=f"hx_{name}{cc}", bufs=2)
            dma(xt[:], xT_dram[cc * 128:(cc + 1) * 128, :])
            t1 = pool.tile([128, L], bf16, tag="hscrb", name=f"hs1_{name}{cc}", bufs=2)
            with nc.allow_low_precision(reason="LN1 xhat in bf16; tol 2e-2"):
                nc.vector.tensor_tensor(t1[:], xt[:], mu_bc[:], SUB)
            t2 = pool.tile([128, L], bf16, tag="hscrb", name=f"hs2_{name}{cc}", bufs=2)
            nc.vector.tensor_tensor(t2[:], t1[:], rstd_bc[:], MUL)
            hb = pool.tile([128, L], bf16, tag=f"hTb_{name}{cc}", name=f"hTb_{name}{cc}")
            nc.scalar.activation(hb[:], t2[:], AF.Identity,
                                 bias=sh_msa[:, cc:cc + 1], scale=s1_msa[:, cc:cc + 1])
            outs.append(hb)
        return outs

    # ---------------- S2: LN1 + modulate (pre and row domains) ------------
    mu_p, rstd_p = ln_stats(xstat_tiles["x_pre"], "p", work)
    hT_pre = make_hT(T["xT_pre"], mu_p, rstd_p, "p", work)
    mu_r, rstd_r = ln_stats(None, "r", work, x_dram=T["x_row"])
    hT_row = make_hT(T["xT_row"], mu_r, rstd_r, "r", work)

    Wxi = load_cast(T["W_in_xi"], DIM, DI, "Wxi")          # 2 x (128,512)
    Wz = load_cast(T["W_in_z"], DIM, DI, "Wz")
    Wxp = load_cast(T["W_xp"], DI, 144, "Wxp")             # 4 x (128,144)
    Wdt = load_cast(T["W_dtm"], DTR, DI, "Wdt")            # 1 x (16,512)
    ysel_b = load_cast(T["ysel"], 128, 64, "ysel", dest=wp)[0]

    smalls = {}
    for nm in ("b_in_xi", "b_in_z", "convw", "convb", "dtb", "Dp_c",
               "lnw_c", "lnb_c", "b_out_c", "b_fc1_c", "b_fc2_c"):
        tt = persist.tile(list(T[nm].shape), f32, tag=nm, name=nm)
        dma(tt[:], T[nm][:, :])
        smalls[nm] = tt

    # conv diag weight tiles: diag[j,tap] = diag(convw[:, j*9+tap])
    cdiag = []
    for j in range(4):
        row = []
        for tap in range(9):
            dg = work.tile([128, 128], bf16, tag=f"cdiag{j}_{tap}", name=f"cdiag{j}_{tap}")
            nc.vector.tensor_scalar_mul(dg[:], identb[:],
                                        smalls["convw"][:, j * 9 + tap:j * 9 + tap + 1])
            row.append(dg)
        cdiag.append(row)

    alog = work.tile([128, 256], f32, tag="stage", name="alog", bufs=2)
    dma(alog[:], T["a_logcols"][:, :])
    acols = wp.tile([128, 256], f32, tag="acols", name="acols")
    nc.scalar.activation(acols[:], alog[:], AF.Exp)
    dpdiag = []
    for jj in range(4):
        dpd = wp.tile([128, 64], bf16, tag=f"dpd{jj}", name=f"dpd{jj}")
        nc.vector.tensor_scalar_mul(dpd[:], ysel_b[:], smalls["Dp_c"][:, jj:jj + 1])
        dpdiag.append(dpd)

    gb_out = persist.tile([128, 2], f32, tag="gb_out", name="gb_out")
    nc.vector.tensor_tensor(gb_out[:], g_msa[:], smalls["b_out_c"][:], MUL)
    gb_fc2 = persist.tile([128, 2], f32, tag="gb_fc2", name="gb_fc2")
    nc.vector.tensor_tensor(gb_fc2[:], g_mlp[:], smalls["b_fc2_c"][:], MUL)

    # ---------------- S3: xi projection into padded conv planes -----------
    # pad[j]: (128, PADW) bf16, image interior at [1+h, 1+w] (34-wide rows)
    pads = []
    for j in range(4):
        pad = work.tile([128, PADW], bf16, tag=f"pad{j}", name=f"pad{j}")
        nc.gpsimd.memset(pad[:], 0.0)
        pads.append(pad)

    for j in range(4):
        ppm = pp.tile([128, L], f32, tag="projp", name=f"pp_xiT{j}", bufs=2)
        for kk in range(2):
            for th in range(2):
                nc.tensor.matmul(
                    ppm[:, th * 512:(th + 1) * 512],
                    Wxi[kk][:, j * 128:(j + 1) * 128],
                    hT_pre[kk][:, th * 512:(th + 1) * 512],
                    start=(kk == 0), stop=(kk == 1))
        pad3 = pads[j][:, :34 * 34].rearrange("p (H W) -> p H W", H=34, W=34)
        nc.scalar.activation(pad3[:, 1:33, 1:33],
                             ppm[:, :].rearrange("p (h w) -> p h w", h=32, w=32),
                             AF.Identity, bias=smalls["b_in_xi"][:, j:j + 1])

    # z projection (row domain) -> spill to DRAM for the post phase
    for j in range(4):
        ppz = pp.tile([128, L], f32, tag="projp", name=f"pp_zTs{j}", bufs=2)
        for kk in range(2):
            for th in range(2):
                nc.tensor.matmul(
                    ppz[:, th * 512:(th + 1) * 512],
                    Wz[kk][:, j * 128:(j + 1) * 128],
                    hT_row[kk][:, th * 512:(th + 1) * 512],
                    start=(kk == 0), stop=(kk == 1))
        zt = work.tile([128, L], bf16, tag="zTs", name=f"zTs{j}", bufs=2)
        nc.scalar.activation(zt[:], ppz[:], AF.Identity, bias=smalls["b_in_z"][:, j:j + 1])
        dma(T["z_spill"][j * 128:(j + 1) * 128, :], zt[:])

    # ---------------- S4: depthwise conv 3x3 via PE diag matmuls ----------
    xsT = []
    HHALF = 16
    HFREE = HHALF * 34
    for j in range(4):
        xs = wp.tile([128, L], bf16, tag=f"xsT{j}", name=f"xsT{j}")
        for hh in range(2):
            base = hh * HHALF * 34
            cvp = pp.tile([128, HFREE], f32, tag="convp", name=f"convp{j}_{hh}", bufs=2)
            for c0 in range(0, HFREE, 512):
                c1 = min(c0 + 512, HFREE)
                for tap in range(9):
                    dy, dx = tap // 3, tap % 3
                    off = base + dy * 34 + dx
                    nc.tensor.matmul(
                        cvp[:, c0:c1], cdiag[j][tap][:],
                        pads[j][:, off + c0:off + c1],
                        start=(tap == 0), stop=(tap == 8))
            nc.scalar.activation(
                xs[:, hh * HHALF * 32:(hh + 1) * HHALF * 32]
                  .rearrange("p (h w) -> p h w", h=HHALF, w=32),
                cvp[:, :].rearrange("p (h w) -> p h w", h=HHALF, w=34)[:, :, 0:32],
                AF.Silu, bias=smalls["convb"][:, j:j + 1])
        xsT.append(xs)

    # ---------------- S5: x_dbl = [B; C] and dt_r -------------------------
    bc_t = wp.tile([128, L], bf16, tag="bc_t", name="bc_t")
    ppbc = pp.tile([128, L], f32, tag="projp", name="ppbc", bufs=2)
    for kk in range(4):
        for th in range(2):
            nc.tensor.matmul(ppbc[:, th * 512:(th + 1) * 512],
                             Wxp[kk][:, 0:128],
                             xsT[kk][:, th * 512:(th + 1) * 512],
                             start=(kk == 0), stop=(kk == 3))
    nc.scalar.copy(bc_t[:], ppbc[:])
    dtr_t = work.tile([16, L], bf16, tag="dtr_t", name="dtr_t")
    ppdtr = pp.tile([16, L], f32, tag="projp", name="ppdtr", bufs=2)
    for kk in range(4):
        for th in range(2):
            nc.tensor.matmul(ppdtr[:, th * 512:(th + 1) * 512],
                             Wxp[kk][:, 128:144],
                             xsT[kk][:, th * 512:(th + 1) * 512],
                             start=(kk == 0), stop=(kk == 3))
    nc.scalar.copy(dtr_t[:], ppdtr[:])

    # ---------------- S6: dt, -dt broadcast, w broadcast ------------------
    wbc, ndtbc = [], []
    bcast_srcs = []
    for j in range(4):
        ppd = pp.tile([128, L], f32, tag="projp", name=f"ppdt{j}", bufs=2)
        for th in range(2):
            nc.tensor.matmul(ppd[:, th * 512:(th + 1) * 512],
                             Wdt[0][:, j * 128:(j + 1) * 128],
                             dtr_t[:, th * 512:(th + 1) * 512],
                             start=True, stop=True)
        spx = work.tile([128, L], f32, tag="spx", name=f"spx{j}", bufs=2)
        nc.scalar.activation(spx[:], ppd[:], AF.Exp, bias=smalls["dtb"][:, j:j + 1])
        dt_b = work.tile([128, L], bf16, tag="dtT", name=f"dtT{j}", bufs=2)
        nc.scalar.activation(dt_b[:], spx[:], AF.Ln, bias=1.0)
        ndt = wp.tile([128, L], bf16, tag="ndtT", name=f"ndtT{j}", bufs=2)
        nc.vector.tensor_scalar_mul(ndt[:], dt_b[:], -1.0)
        w_b = wp.tile([128, L], bf16, tag="wTtmp", name=f"wT{j}", bufs=2)
        nc.vector.tensor_tensor(w_b[:], dt_b[:], xsT[j][:], MUL)
        for par in range(2):
            g = 2 * j + par
            wb = wp.tile([128, L], bf16, tag=f"wbc{g}", name=f"wbc{g}")
            wbc.append(wb)
            nb = wp.tile([128, L], bf16, tag=f"ndtbc{g}", name=f"ndtbc{g}")
            ndtbc.append(nb)
            bcast_srcs.append((wb, w_b, nb, ndt, par))

    pre_ps.close()
    workstack.close()

    # ---------------- post-phase pools (opened early: tiles live across) --
    post = ExitStack()
    pf = post.enter_context(tc.tile_pool(name="postf", bufs=1))

    ones_f = pf.tile([128, 1], f32, tag="ones_f", name="ones_f")
    nc.gpsimd.memset(ones_f[:], 1.0)
    ones_b = pf.tile([128, 1], bf16, tag="ones_b", name="ones_b")
    nc.gpsimd.memset(ones_b[:], 1.0)

    siluz = []
    for j in range(4):
        zrl = pf.tile([128, L], bf16, tag="zrl", name=f"zrl{j}", bufs=2)
        dma(zrl[:], T["z_spill"][j * 128:(j + 1) * 128, :])
        sz = pf.tile([128, L], bf16, tag="siluz", name=f"siluz{j}", bufs=4)
        nc.scalar.activation(sz[:], zrl[:], AF.Silu)
        siluz.append(sz)

    # ---------------- S7: scan (g-major) + chunked AllGather + combine ----
    scan_st = ExitStack()
    yps = scan_st.enter_context(tc.tile_pool(name="yps", bufs=1, space="PSUM"))
    spool = scan_st.enter_context(tc.tile_pool(name="spool", bufs=2))
    bpool = scan_st.enter_context(tc.tile_pool(name="bpool", bufs=1))

    ssum = yps.tile([128, 8], f32, tag="ssum", name="ssum", bufs=1)
    ssq = yps.tile([128, 8], f32, tag="ssq", name="ssq", bufs=1)
    y_row = [None] * 4
    sq_y = [None] * 4
    yrps = [None] * 4

    # staggered post-weight loads: (dram, r0, r, h0, h1, dest_tile) pairs
    wload_specs = []
    for nm, dram, rows, cols in (("Wout", T["W_out"], DI, DIM),
                                 ("Wfc1", T["W_fc1"], DIM, HID),
                                 ("Wfc2", T["W_fc2"], HID, DIM)):
        n = (rows + 127) // 128
        tiles = []
        for jj in range(n):
            tb = pf.tile([128, cols], bf16, tag=f"{nm}_b{jj}", name=f"{nm}_b{jj}")
            tiles.append(tb)
            for h0 in range(0, cols, 512):
                wload_specs.append((dram, jj * 128, h0, min(h0 + 512, cols), tb))
        if nm == "Wout":
            Wout = tiles
        elif nm == "Wfc1":
            Wfc1 = tiles
        else:
            Wfc2 = tiles
    wload_pending = []
    wload_i = [0]

    def emit_wloads(ndma):
        for tf, tb, h0, h1 in wload_pending:
            nc.scalar.copy(tb[:, h0:h1], tf[:, :h1 - h0])
        wload_pending.clear()
        for _ in range(ndma):
            if wload_i[0] >= len(wload_specs):
                return
            dram, r0, h0, h1, tb = wload_specs[wload_i[0]]
            wload_i[0] += 1
            tf = pf.tile([128, 512], f32, tag="pstage", name=f"pst_w{wload_i[0]}", bufs=2)
            dma(tf[:, :h1 - h0], dram[r0:r0 + 128, h0:h1])
            wload_pending.append((tf, tb, h0, h1))

    def emit_combine(g, anchor=None, mk=None):
        j, par = g // 2, g % 2
        if par == 0:
            y_row[j] = pf.tile([128, L], bf16, tag="yrow", name=f"yrow{j}", bufs=4)
            sq_y[j] = pf.tile([128, L], bf16, tag="sqy", name=f"sq_y{j}", bufs=2)
        yr, sq = y_row[j], sq_y[j]
        r0, r1 = par * 64, par * 64 + 64
        ysk = []
        for k in range(4):
            t16 = pf.tile([64, L], bf16, tag="ysk", name=f"ysk{g}_{k}", bufs=4)
            ld = dma(t16[:], T[f"ys_g{g}"][k, :, :])
            if anchor is not None:
                _add_dep_helper(ld.ins, anchor.ins, sync=True,
                                reason="defer gather read past next group")
            ysk.append(t16)
        yrp = yrps[j]
        views = [
            ysk[0][:, :],
            (ysk[1][:, :].rearrange("p (w h) -> p w h", w=32, h=32)
                         .rearrange("p w h -> p h w")),
            ysk[2][:, ::-1],
            (ysk[3][:, ::-1].rearrange("p (w h) -> p w h", w=32, h=32)
                            .rearrange("p w h -> p h w")),
        ]
        for th in range(2):
            for k in range(4):
                v = views[k]
                if len(v.shape) == 3:
                    vth = v[:, th * 16:(th + 1) * 16, :]
                else:
                    vth = v[:, th * 512:(th + 1) * 512]
                nc.tensor.matmul(
                    yrp[r0:r1, th * 512:(th + 1) * 512],
                    identb[0:64, 0:64], vth,
                    start=(k == 0), stop=(k == 3))
        if mk is not None:
            nc.scalar.activation(yr[r0:r1, :], yrp[r0:r1, :], AF.Identity,
                                 bias=mk[r0:r1, 0:1])
            nc.scalar.activation(sq[r0:r1, :], yrp[r0:r1, :], AF.Square,
                                 bias=mk[r0:r1, 0:1])
        else:
            nc.scalar.copy(yr[r0:r1, :], yrp[r0:r1, :])
            nc.scalar.activation(sq[r0:r1, :], yrp[r0:r1, :], AF.Square)
        for tb in range(8):
            nc.tensor.matmul(ssum[:, tb:tb + 1],
                             yr[r0:r1, tb * 128:(tb + 1) * 128],
                             ones_b[r0:r1, :],
                             start=(g == 0), stop=(g == 7))
            nc.tensor.matmul(ssq[:, tb:tb + 1],
                             sq[r0:r1, tb * 128:(tb + 1) * 128],
                             ones_b[r0:r1, :],
                             start=(g == 0), stop=(g == 7))

    it = 0
    pending_g = []
    ys16_insts = [None] * 8
    mk_tiles = [None] * 8
    from concourse.bass import _add_dep_helper

    def emit_bcast(g):
        wb, w_b, nb, ndt, par = bcast_srcs[g]
        wsrc = w_b[par * 64:par * 64 + 64, :]
        dma(wb[0:64, :], wsrc)
        dma(wb[64:128, :], wsrc)
        nsrc = ndt[par * 64:par * 64 + 64, :]
        dma(nb[0:64, :], nsrc)
        dma(nb[64:128, :], nsrc)
    for g in range(2):
        emit_bcast(g)

    def scan_it(j, par, ch, il, ypt, Bb, Cb):
        nonlocal_it = scan_it.it
        g = 2 * j + par
        i = ch * CHUNK + il
        dA = spool.tile([128, L], bf16, tag="dA", name=f"dA{nonlocal_it}", bufs=3)
        nc.scalar.activation(dA[:], ndtbc[g][:], AF.Exp,
                             scale=acols[:, g * 32 + i:g * 32 + i + 1])
        if ch == 1 and il == 0:
            mkt = spool.tile([128, 1], bf16, tag="mk", name=f"mk{g}", bufs=8)
            nc.scalar.activation(mkt[:, 0:1], dA[:, 0:1], AF.Identity, scale=0.0)
            mk_tiles[g] = mkt
        xin = spool.tile([128, L], bf16, tag="xin", name=f"xin{nonlocal_it}", bufs=3)
        xeng = nc.vector if (nonlocal_it % 7 == 0) else nc.gpsimd
        xeng.tensor_tensor(xin[:], wbc[g][:], Bb[il][:], MUL)
        h = spool.tile([128, L], bf16, tag="h", name=f"h{nonlocal_it}", bufs=3)
        nc.vector.tensor_tensor_scan(h[:], dA[:], xin[:], 0.0, MUL, ADD)
        yc = spool.tile([128, L], bf16, tag="yc", name=f"yc{nonlocal_it}", bufs=3)
        nc.vector.tensor_tensor(yc[:], h[:], Cb[il][:], MUL)
        for th in range(2):
            nc.tensor.matmul(
                ypt[par * 64:par * 64 + 64, th * 512:(th + 1) * 512],
                ysel_b[:], yc[:, th * 512:(th + 1) * 512],
                start=(ch == 0 and il == 0), stop=False)
        scan_it.it += 1
    scan_it.it = 0

    def load_bc(j, ch, sfx=""):
        Bb, Cb = [], []
        for il in range(CHUNK):
            i = ch * CHUNK + il
            bbt = bpool.tile([128, L], bf16, tag="Bb", name=f"Bb{sfx}{j}_{ch}_{il}",
                             bufs=2 * CHUNK - 2)
            dma(bbt[:], bc_t[2 * i:2 * i + 2, :]
                .partition_broadcast(64).rearrange("d n f -> n d f"))
            Bb.append(bbt)
            cbt = bpool.tile([128, L], bf16, tag="Cb", name=f"Cb{sfx}{j}_{ch}_{il}",
                             bufs=2 * CHUNK - 2)
            dma(cbt[:], bc_t[64 + 2 * i:64 + 2 * i + 2, :]
                .partition_broadcast(64).rearrange("d n f -> n d f"))
            Cb.append(cbt)
        return Bb, Cb

    def emit_tail(j, par, ypt):
        g = 2 * j + par
        r0, r1 = par * 64, par * 64 + 64
        for th in range(2):
            nc.tensor.matmul(
                ypt[r0:r1, th * 512:(th + 1) * 512],
                dpdiag[j][r0:r1, :], xsT[j][r0:r1, th * 512:(th + 1) * 512],
                start=False, stop=(th == 1))
        ysb16 = spool.tile([64, L], bf16, tag="ys16", name=f"ys16_{g}", bufs=1)
        ys16_insts[g] = nc.scalar.copy(ysb16[:], ypt[r0:r1, :])
        dma(T["ys_lb"][g * 64:(g + 1) * 64, :], ysb16[:])
        nc.gpsimd.collective_compute(
            "AllGather", BYP,
            replica_groups=[[0, 1, 2, 3], [4, 5, 6, 7]],
            ins=[T["ys_lb"][g * 64:(g + 1) * 64, :]],
            outs=[T[f"ys_g{g}"][:, :, :]],
        )
        pending_g.append(g)

    for j in range(3):
        ypt = yps.tile([128, L], f32, tag="ypt", name=f"ypt{j}", bufs=2)
        yrps[j] = yps.tile([128, L], f32, tag="yrp", name=f"yrp{j}", bufs=1)
        if j > 0:
            emit_bcast(2 * j)
            emit_bcast(2 * j + 1)
        for par in range(2):
            for ch in range(NCHUNK):
                if ch in (2, 5) and len(pending_g) > 1:
                    gq = pending_g.pop(0)
                    emit_combine(gq, anchor=ys16_insts[gq + 1],
                                 mk=mk_tiles[gq + 2] if gq + 2 < 8 else None)
                if ch in (3, 6):
                    emit_wloads(1)
                Bb, Cb = load_bc(j, ch, sfx=f"q{par}_")
                for il in range(CHUNK):
                    scan_it(j, par, ch, il, ypt, Bb, Cb)
            emit_tail(j, par, ypt)

    # last j: g-major so the two final gathers stagger
    emit_bcast(6)
    emit_bcast(7)
    j = 3
    ypt = yps.tile([128, L], f32, tag="ypt", name=f"ypt{j}", bufs=2)
    yrps[j] = yps.tile([128, L], f32, tag="yrp", name=f"yrp{j}", bufs=1)
    for par in range(2):
        for ch in range(NCHUNK):
            if ch in (2, 5) and len(pending_g) > 1:
                gq = pending_g.pop(0)
                emit_combine(gq, anchor=ys16_insts[gq + 1],
                             mk=mk_tiles[gq + 2] if gq + 2 < 8 else None)
            if ch in (3, 6):
                emit_wloads(1)
            Bb, Cb = load_bc(j, ch, sfx=f"p{par}_")
            for il in range(CHUNK):
                scan_it(j, par, ch, il, ypt, Bb, Cb)
        emit_tail(j, par, ypt)

    while pending_g:
        emit_wloads(0)
        gq = pending_g.pop(0)
        emit_combine(gq, anchor=ys16_insts[gq + 1] if gq + 1 < 8 else None,
                     mk=mk_tiles[gq + 2] if gq + 2 < 8 else None)
    emit_wloads(16)
    emit_wloads(0)

    # ---------------- S12: LN over d + ln_w/ln_b, gate --------------------
    def stat_cols(ssum_t, ssq_t, dim, name):
        mu = pf.tile([128, 8], f32, tag="pmu", name=f"pmu_{name}", bufs=2)
        nc.vector.tensor_scalar_mul(mu[:], ssum_t[:], 1.0 / dim)
        mu2 = pf.tile([128, 8], f32, tag="pmu2", name=f"pmu2_{name}", bufs=2)
        nc.vector.tensor_tensor(mu2[:], mu[:], mu[:], MUL)
        var = pf.tile([128, 8], f32, tag="pvar", name=f"pvar_{name}", bufs=2)
        nc.vector.scalar_tensor_tensor(var[:], ssq_t[:], 1.0 / dim, mu2[:], MUL, SUB)
        std = pf.tile([128, 8], f32, tag="pstd", name=f"pstd_{name}", bufs=2)
        nc.scalar.activation(std[:], var[:], AF.Sqrt, bias=eps_col[:, 0:1])
        rstd = pf.tile([128, 8], f32, tag="prstd", name=f"prstd_{name}", bufs=2)
        nc.vector.reciprocal(rstd[:], std[:])
        return mu, rstd

    mu_y, rstd_y = stat_cols(ssum, ssq, DI, "y")

    scan_st.close()

    ppost = post.enter_context(tc.tile_pool(name="ps_post", bufs=2, space="PSUM"))
    ymu_bc = bcast_cols(mu_y, "Pym", pf, "bcA", ppost, "php", dt_out=bf16)
    yrstd_bc = bcast_cols(rstd_y, "Pyr", pf, "bcB", ppost, "php", dt_out=bf16)

    gated = []
    for j in range(4):
        t1 = pf.tile([128, L], bf16, tag="psc", name=f"lny1_{j}", bufs=2)
        nc.vector.tensor_tensor(t1[:], y_row[j][:], ymu_bc[:], SUB)
        t2 = pf.tile([128, L], bf16, tag="psc", name=f"lny2_{j}", bufs=2)
        nc.vector.tensor_tensor(t2[:], t1[:], yrstd_bc[:], MUL)
        t3 = pf.tile([128, L], bf16, tag="psc", name=f"lny3_{j}", bufs=2)
        nc.scalar.activation(t3[:], t2[:], AF.Identity,
                             bias=smalls["lnb_c"][:, j:j + 1],
                             scale=smalls["lnw_c"][:, j:j + 1])
        gt = pf.tile([128, L], bf16, tag=f"gated{j}", name=f"gated{j}")
        geng = nc.gpsimd if (j % 2 == 1) else nc.vector
        geng.tensor_tensor(gt[:], t3[:], siluz[j][:], MUL)
        gated.append(gt)

    # ---------------- S14: hy, residual -----------------------------------
    xTrs = []
    for cc in range(2):
        xt = pf.tile([128, L], f32, tag="xTrl", name=f"xTr{cc}", bufs=2)
        dma(xt[:], T["xT_row"][cc * 128:(cc + 1) * 128, :])
        xTrs.append(xt)
    x2T = []
    for cc in range(2):
        php = ppost.tile([128, L], f32, tag="php", name=f"php{cc}", bufs=2)
        for kk in range(4):
            for th in range(2):
                nc.tensor.matmul(php[:, th * 512:(th + 1) * 512],
                                 Wout[kk][:, cc * 128:(cc + 1) * 128],
                                 gated[kk][:, th * 512:(th + 1) * 512],
                                 start=(kk == 0), stop=(kk == 3))
        t1 = pf.tile([128, L], f32, tag="pscf", name=f"hyg{cc}", bufs=2)
        nc.scalar.activation(t1[:], php[:], AF.Identity,
                             bias=gb_out[:, cc:cc + 1], scale=g_msa[:, cc:cc + 1])
        x2 = pf.tile([128, L], bf16, tag=f"x2T{cc}", name=f"x2T{cc}")
        with nc.allow_low_precision(reason="x2 residual in bf16; tol 2e-2"):
            nc.vector.tensor_tensor(x2[:], t1[:], xTrs[cc][:], ADD)
        x2T.append(x2)

    # ---------------- S15: LN2 + modulate ---------------------------------
    x2sum = ppost.tile([128, 8], f32, tag="pfc2p", name="x2sum", bufs=2)
    x2sq = ppost.tile([128, 8], f32, tag="pfc2p", name="x2sq", bufs=2)
    sqx = []
    for ccj in range(2):
        sq = pf.tile([128, L], bf16, tag="sqt", name=f"sq_x2{ccj}", bufs=2)
        nc.scalar.activation(sq[:], x2T[ccj][:], AF.Square)
        sqx.append(sq)
    for tb in range(8):
        for ccj in range(2):
            nc.tensor.matmul(x2sum[:, tb:tb + 1],
                             x2T[ccj][:, tb * 128:(tb + 1) * 128], ones_b[:],
                             start=(ccj == 0), stop=(ccj == 1))
            nc.tensor.matmul(x2sq[:, tb:tb + 1],
                             sqx[ccj][:, tb * 128:(tb + 1) * 128], ones_b[:],
                             start=(ccj == 0), stop=(ccj == 1))
    mu_x2, rstd_x2 = stat_cols(x2sum, x2sq, DIM, "x2")
    x2mu_bc = bcast_cols(mu_x2, "Px2m", pf, "bcA", ppost, "php", dt_out=bf16)
    x2rstd_bc = bcast_cols(rstd_x2, "Px2r", pf, "bcB", ppost, "php", dt_out=bf16)
    mT = []
    for cc in range(2):
        t1 = pf.tile([128, L], bf16, tag="psc", name=f"m1_{cc}", bufs=2)
        nc.vector.tensor_tensor(t1[:], x2T[cc][:], x2mu_bc[:], SUB)
        t2 = pf.tile([128, L], bf16, tag="psc", name=f"m2_{cc}", bufs=2)
        nc.vector.tensor_tensor(t2[:], t1[:], x2rstd_bc[:], MUL)
        mb = pf.tile([128, L], bf16, tag=f"mT{cc}", name=f"mT{cc}")
        nc.scalar.activation(mb[:], t2[:], AF.Identity,
                             bias=sh_mlp[:, cc:cc + 1], scale=s1_mlp[:, cc:cc + 1])
        mT.append(mb)

    # ---------------- S16: MLP + final residual ---------------------------
    pfc2ps = [ppost.tile([128, L], f32, tag="pfc2p", name=f"pfc2_{cc}", bufs=2)
              for cc in range(2)]
    for kk in range(8):
        pfc = ppost.tile([128, L], f32, tag="php", name=f"pfc1_{kk}", bufs=2)
        for kk2 in range(2):
            for th in range(2):
                nc.tensor.matmul(pfc[:, th * 512:(th + 1) * 512],
                                 Wfc1[kk2][:, kk * 128:(kk + 1) * 128],
                                 mT[kk2][:, th * 512:(th + 1) * 512],
                                 start=(kk2 == 0), stop=(kk2 == 1))
        gl = pf.tile([128, L], bf16, tag="gelu", name=f"gelu{kk}", bufs=2)
        nc.scalar.activation(gl[:], pfc[:], AF.Gelu_apprx_tanh, bias=smalls["b_fc1_c"][:, kk:kk + 1])
        for cc in range(2):
            for th in range(2):
                nc.tensor.matmul(pfc2ps[cc][:, th * 512:(th + 1) * 512],
                                 Wfc2[kk][:, cc * 128:(cc + 1) * 128],
                                 gl[:, th * 512:(th + 1) * 512],
                                 start=(kk == 0), stop=(kk == 7))

    for cc in range(2):
        t1 = pf.tile([128, L], f32, tag="pscf", name=f"mlpg{cc}", bufs=2)
        nc.scalar.activation(t1[:], pfc2ps[cc][:], AF.Identity,
                             bias=gb_fc2[:, cc:cc + 1], scale=g_mlp[:, cc:cc + 1])
        o = pf.tile([128, L], f32, tag="outTt", name=f"outT{cc}", bufs=1)
        nc.vector.tensor_tensor(o[:], t1[:], x2T[cc][:], ADD)
        dma(T["outT"][cc * 128:(cc + 1) * 128, :], o[:])

    post.close()
    wstack.close()
    perstack.close()


# ---------------------------------------------------------------------------
# Host side
_PROGRAM = None


def _get_program():
    global _PROGRAM
    if _PROGRAM is None:
        _PROGRAM = build_program()
    return _PROGRAM


def _q_img(x, k):
    img = x.reshape(Hs, Ws, -1)
    if k == 0:
        out = img
    elif k == 1:
        out = img.transpose(1, 0, 2)
    elif k == 2:
        out = img[::-1, ::-1]
    else:
        out = img.transpose(1, 0, 2)[::-1, ::-1]
    return np.ascontiguousarray(out.reshape(L, -1))


def _conv_w_q(w, k):
    if k == 0:
        return w
    if k == 1:
        return np.ascontiguousarray(w.transpose(1, 0, 2))
    if k == 2:
        return np.ascontiguousarray(w[::-1, ::-1])
    return np.ascontiguousarray(w.transpose(1, 0, 2)[::-1, ::-1])


def _col128(v, ncols):
    return np.ascontiguousarray(v.reshape(ncols, 128).T)


def prep_inputs(inputs):
    inp = {k: np.asarray(v, dtype=np.float32) for k, v in inputs.items()}
    x, c = inp["x"], inp["c"]

    shared = {}
    shared["W_ada"] = inp["W_ada"]
    shared["b_ada"] = inp["b_ada"].reshape(1, 6 * DIM)
    W_in = inp["W_in"]
    shared["W_in_xi"] = np.ascontiguousarray(W_in[:, :DI])
    shared["W_in_z"] = np.ascontiguousarray(W_in[:, DI:])
    b_in = inp["b_in"]
    shared["b_in_xi"] = _col128(b_in[:DI], 4)
    shared["b_in_z"] = _col128(b_in[DI:], 4)
    shared["lnw_c"] = _col128(inp["ln_w"], 4)
    shared["lnb_c"] = _col128(inp["ln_b"], 4)
    shared["W_out"] = inp["W_out"]
    shared["b_out_c"] = _col128(inp["b_out"], 2)
    shared["W_fc1"] = inp["W_fc1"]
    shared["b_fc1_c"] = _col128(inp["b_fc1"], 8)
    shared["W_fc2"] = inp["W_fc2"]
    shared["b_fc2_c"] = _col128(inp["b_fc2"], 2)
    shared["ident"] = np.eye(128, dtype=np.float32)
    p = np.arange(128)
    ys = np.zeros((128, 64), np.float32)
    ys[p, p % 64] = 1.0
    shared["ysel"] = ys

    in_maps = []
    for core in range(8):
        b, k = core // 4, core % 4
        m = dict(shared)
        xb = x[b]
        xpre = _q_img(xb, k)
        m["x_pre"] = xpre
        m["xT_pre"] = np.ascontiguousarray(xpre.T)
        m["x_row"] = np.ascontiguousarray(xb)
        m["xT_row"] = np.ascontiguousarray(xb.T)
        m["c_vec"] = c[b].reshape(1, DIM)

        cw = _conv_w_q(inp["conv_w"].reshape(3, 3, DI), k)
        cwt = cw.reshape(9, 4, 128)                       # [tap, dtile, p]
        m["convw"] = np.ascontiguousarray(cwt.transpose(2, 1, 0).reshape(128, 36))
        m["convb"] = _col128(inp["conv_b"], 4)

        Wxp = inp["W_xproj"][k]                           # (DI, 144) cols [dtr,B,C]
        m["W_xp"] = np.ascontiguousarray(
            np.concatenate([Wxp[:, DTR:DTR + DS], Wxp[:, DTR + DS:], Wxp[:, :DTR]],
                           axis=1))
        m["W_dtm"] = np.ascontiguousarray(inp["W_dt"][k])
        m["dtb"] = _col128(inp["dt_bias"][k], 4)
        m["Dp_c"] = _col128(inp["Dp"][k], 4)

        alog = inp["A_log"][k]                            # (DI, DS)
        acols = np.zeros((128, 256), np.float32)
        for g in range(8):
            for i in range(NPAIRS):
                acols[:, g * 32 + i] = alog[g * 64 + (p % 64), 2 * i + (p // 64)]
        m["a_logcols"] = acols
        in_maps.append(m)
    return in_maps


def kernel(**inputs):
    nc = _get_program()
    in_maps = prep_inputs(inputs)
    res = run_bass_kernel_spmd(nc, in_maps, list(range(8)))
    out = np.zeros((B, L, DIM), np.float32)
    for b in range(B):
        out[b] = res.results[4 * b]["outT"].T
    return out
